# revision 1
# baseline (speedup 1.0000x reference)
"""Trainium2 Bass kernel for the rumor-GCN masked-autoencoder loss.

Strategy (8 NeuronCores, SPMD single NEFF):
  - Nodes are partitioned into 8 contiguous ranges (25000 each). Each core
    aggregates the in-edges of its own nodes (TD: grouped by dst, BU: grouped
    by src) -- "halo exchange" is done by the host pre-gathering the x-rows
    each core needs, so layer-1 is fully local.
  - Layer-1 linear is one fused [512->512] bf16 matmul over each core's
    needed-node set (4 GCN heads side by side). GCN symmetric norms are
    folded into per-row scales (dinv[src] into z at the P1 copy-out, dinv[dst]
    at aggregation finalize), so the sparse selection matrices are pure 0/1.
  - Edge aggregation: dma_gather pulls source rows into K-tiles of 128 edge
    slots; a one-hot S matrix per K-tile turns scatter-add into PE matmuls
    accumulating in PSUM per 128-dst-node block.
  - Layer-2 inputs (z2) are AllGathered across cores, then the same gather/
    matmul machinery runs against global 32K-row windows (int16 gather idx).
  - global_add_pool via matmuls into a persistent PSUM accumulator; the
    pooled sums + the masked-node cosine partial sum go through one small
    AllReduce; every core then computes the identical final scalar loss.
"""

import sys

import numpy as np

sys.path.insert(0, "/opt/trn_rl_repo")

# ---------------------------------------------------------------- config

class Cfg:
    def __init__(self, N, E, G, M, C=8, WIN=32768, GB=16, GB2=8, NF=2048):
        self.N, self.E, self.G, self.M, self.C = N, E, G, M, C
        self.IN, self.HID, self.OUT = 512, 128, 64
        self.WIN, self.GB, self.GB2, self.NF = WIN, GB, GB2, NF
        assert N % C == 0
        self.OWN = N // C
        self.NB = -(-self.OWN // 128)
        self.OWNP = self.NB * 128
        self.NPAD = C * self.OWNP
        self.NW2 = -(-self.NPAD // WIN)


FULL = Cfg(N=200000, E=400000, G=128, M=100000)

_WNAMES = [p + s for p in ("on_td", "on_bu", "tgt_td", "tgt_bu")
           for s in ("_W1", "_b1", "_W2", "_b2")]


def _rep16(idx_flat, nslots):
    """int16 index list -> [128, nslots//16] layout (16-part wrap, 8x replicated)."""
    blk = np.zeros((16, nslots // 16), dtype=np.int16)
    k = np.arange(len(idx_flat))
    blk[k % 16, k // 16] = idx_flat
    return np.tile(blk, (8, 1))


def _bcast(vec, parts=128):
    return np.broadcast_to(np.asarray(vec)[None, :], (parts, len(vec))).copy()


# ---------------------------------------------------------------- host prep

def host_prep(cfg, inp):
    import ml_dtypes
    bf16 = ml_dtypes.bfloat16
    c = cfg
    x = np.asarray(inp["x"], np.float32)
    token = np.asarray(inp["enc_mask_token"], np.float32).reshape(-1)
    ei = np.asarray(inp["edge_index"])
    src, dst = ei[0].astype(np.int64), ei[1].astype(np.int64)
    batch = np.asarray(inp["batch"]).astype(np.int64)
    mask_nodes = np.asarray(inp["mask_nodes"]).astype(np.int64)
    W = {k: np.asarray(inp[k], np.float32) for k in _WNAMES}

    dinv_td = (1.0 / np.sqrt(np.bincount(dst, minlength=c.N) + 1.0)).astype(np.float32)
    dinv_bu = (1.0 / np.sqrt(np.bincount(src, minlength=c.N) + 1.0)).astype(np.float32)
    is_masked = np.zeros(c.N, bool)
    is_masked[mask_nodes] = True
    xbf = x.astype(bf16)

    # ---- per-core edge lists (agg_dst local block/lane, agg_src global)
    # dir 0 = TD (aggregate src -> dst, dinv_td); dir 1 = BU (dst -> src, dinv_bu)
    core_edges = []   # [core][dir] -> (adst_local, asrc_global)
    for ci in range(c.C):
        lo, hi = ci * c.OWN, (ci + 1) * c.OWN
        per = []
        for d in range(2):
            ad, as_ = (dst, src) if d == 0 else (src, dst)
            sel = (ad >= lo) & (ad < hi)
            adst = ad[sel] - lo
            asrc = as_[sel]
            # self loops
            adst = np.concatenate([adst, np.arange(c.OWN, dtype=np.int64)])
            asrc = np.concatenate([asrc, np.arange(lo, hi, dtype=np.int64)])
            per.append((adst, asrc))
        core_edges.append(per)

    # ---- needed sets / z-row maps
    RU, RK = [], []
    needed_um, needed_mk, localmap = [], [], []
    for ci in range(c.C):
        lo, hi = ci * c.OWN, (ci + 1) * c.OWN
        nodes = np.unique(np.concatenate(
            [np.arange(lo, hi, dtype=np.int64),
             core_edges[ci][0][1], core_edges[ci][1][1]]))
        um = nodes[~is_masked[nodes]]
        mk = nodes[is_masked[nodes]]
        needed_um.append(um); needed_mk.append(mk)
        RU.append(len(um)); RK.append(len(mk))
    RU_PAD = -(-max(RU) // 128) * 128
    RK_PAD = -(-max(RK) // 128) * 128
    RT = RU_PAD + RK_PAD
    NW1 = -(-RT // c.WIN)
    for ci in range(c.C):
        lm = np.full(c.N, -1, np.int64)
        lm[needed_um[ci]] = np.arange(RU[ci])
        lm[needed_mk[ci]] = RU_PAD + np.arange(RK[ci])
        localmap.append(lm)

    # ---- slot schedules: for (dir, layer) build common KT[b][w], flat kt order
    # flat order: group g -> window w -> block b in g -> tiles
    def build_sched(layer):
        GB = c.GB if layer == 0 else c.GB2
        NG = -(-c.NB // GB)
        scheds = []
        for d in range(2):
            NW = NW1 if layer == 0 else c.NW2
            cnt = np.zeros((c.C, c.NB, NW), np.int64)
            per_core_bwe = []
            for ci in range(c.C):
                adst, asrc = core_edges[ci][d]
                if layer == 0:
                    row = localmap[ci][asrc]
                else:
                    row = (asrc // c.OWN) * c.OWNP + (asrc % c.OWN)
                b = adst // 128
                w = row // c.WIN
                np.add.at(cnt[ci], (b, w), 1)
                per_core_bwe.append((b, w, row - w * c.WIN, adst % 128))
            KT = -(-cnt.max(axis=0) // 128)  # [NB, NW]
            # flat kt offsets in group-major order
            ktoff = np.zeros((c.NB, NW), np.int64)
            acc = 0
            group_meta = []
            for g in range(NG):
                blks = range(g * GB, min((g + 1) * GB, c.NB))
                ops = []
                for w in range(NW):
                    nk = int(sum(KT[b, w] for b in blks))
                    if nk == 0:
                        continue
                    base = acc
                    for b in blks:
                        ktoff[b, w] = acc
                        acc += KT[b, w]
                    ops.append((w, base, nk))
                group_meta.append(ops)
            TOTKT = acc
            scheds.append(dict(NW=NW, KT=KT, ktoff=ktoff, TOTKT=TOTKT,
                               groups=group_meta, cnt=cnt, GB=GB,
                               per_core_bwe=per_core_bwe))
        return scheds

    sched1 = build_sched(0)
    sched2 = build_sched(1)

    def build_slots(sch, per_core_idx):
        """-> (S_host [128, TOTKT*128] bf16, idx [128, TOTKT*8] int16)"""
        b, w, rel, lane = per_core_idx
        KT, ktoff, TOTKT = sch["KT"], sch["ktoff"], sch["TOTKT"]
        # position within (b, w) segment
        order = np.lexsort((np.arange(len(b)), w, b))
        bs, ws, rels, lanes = b[order], w[order], rel[order], lane[order]
        seg = bs * sch["NW"] + ws
        segchange = np.r_[True, seg[1:] != seg[:-1]]
        segstart = np.maximum.accumulate(np.where(segchange, np.arange(len(seg)), 0))
        pos = np.arange(len(seg)) - segstart
        slot = ktoff[bs, ws] * 128 + pos
        nslots = TOTKT * 128
        idx_flat = np.zeros(nslots, np.int64)
        idx_flat[slot] = rels
        S = np.zeros((128, TOTKT * 128), bf16)
        S[slot % 128, (slot // 128) * 128 + lanes] = 1.0
        assert rels.max(initial=0) < 32768
        return S, _rep16(idx_flat.astype(np.int16), nslots)

    # ---- masked-node loss prep
    owner = mask_nodes // c.OWN
    mrows = [mask_nodes[owner == ci] - ci * c.OWN for ci in range(c.C)]
    MP = max(2048, -(-max(len(r) for r in mrows) // 2048) * 2048)

    # ---- shared (replicated) weight inputs
    w1all = np.concatenate([W["on_td_W1"], W["tgt_td_W1"],
                            W["on_bu_W1"], W["tgt_bu_W1"]], axis=1).astype(bf16)
    w2_td = np.concatenate([W["on_td_W2"], W["tgt_td_W2"]], axis=1).astype(bf16)
    w2_bu = np.concatenate([W["on_bu_W2"], W["tgt_bu_W2"]], axis=1).astype(bf16)
    ton = np.concatenate([token @ W["on_td_W1"], token @ W["on_bu_W1"]])
    tonbc = _bcast(ton).astype(bf16)
    b1bc_td = _bcast(np.concatenate([W["on_td_b1"], W["tgt_td_b1"]]))
    b1bc_bu = _bcast(np.concatenate([W["on_bu_b1"], W["tgt_bu_b1"]]))
    b2bc_td = _bcast(np.concatenate([W["on_td_b2"], W["tgt_td_b2"]]))
    b2bc_bu = _bcast(np.concatenate([W["on_bu_b2"], W["tgt_bu_b2"]]))
    ones = np.ones((128, 1), np.float32)
    gmask = np.zeros((128, 1), np.float32)
    gmask[:c.G, 0] = 1.0

    # ---- per-core inputs
    in_maps = []
    for ci in range(c.C):
        lo = ci * c.OWN
        um, mk = needed_um[ci], needed_mk[ci]
        xT = np.zeros((512, RT), bf16)
        xT[:, :len(um)] = xbf[um].T
        xT[:, RU_PAD:RU_PAD + len(mk)] = xbf[mk].T

        def rowarr(dv):
            a = np.ones(RT, np.float32)
            a[:len(um)] = dv[um]
            a[RU_PAD:RU_PAD + len(mk)] = dv[mk]
            return np.ascontiguousarray(a.reshape(-1, 128).T)  # [128, RT//128]

        def dstarr(dv):
            a = np.ones(c.OWNP, np.float32)
            a[:c.OWN] = dv[lo:lo + c.OWN]
            return np.ascontiguousarray(a.reshape(-1, 128).T)  # [128, NB]

        m = dict(xT=xT,
                 dloc_td=rowarr(dinv_td), dloc_bu=rowarr(dinv_bu),
                 ddst_td=dstarr(dinv_td), ddst_bu=dstarr(dinv_bu))
        for d, nm in ((0, "td"), (1, "bu")):
            S, idx = build_slots(sched1[d], sched1[d]["per_core_bwe"][ci])
            m[f"s_{nm}1"], m[f"i_{nm}1"] = S, idx
            S, idx = build_slots(sched2[d], sched2[d]["per_core_bwe"][ci])
            m[f"s_{nm}2"], m[f"i_{nm}2"] = S, idx
        rows = mrows[ci]
        mi = np.zeros(MP, np.int64); mi[:len(rows)] = rows
        mw = np.zeros(MP, np.float32); mw[:len(rows)] = 1.0
        m["midx"] = _rep16(mi.astype(np.int16), MP)
        m["mw"] = np.ascontiguousarray(mw.reshape(-1, 128).T)  # [128, MP//128]
        pp = np.zeros((128, c.NB * 128), np.float32)
        nid = np.arange(c.OWN)
        pp[nid % 128, (nid // 128) * 128 + batch[lo:lo + c.OWN]] = 1.0
        m["ppool"] = pp
        m.update(w1all=w1all, w2_td=w2_td, w2_bu=w2_bu, tonbc=tonbc,
                 b1bc_td=b1bc_td, b1bc_bu=b1bc_bu, b2bc_td=b2bc_td,
                 b2bc_bu=b2bc_bu, ones=ones, gmask=gmask)
        in_maps.append(m)

    meta = dict(RT=RT, RU_PAD=RU_PAD, RK_PAD=RK_PAD, NW1=NW1, MP=MP,
                sched1=sched1, sched2=sched2)
    return meta, in_maps


# ---------------------------------------------------------------- program

def build_program(cfg, meta):
    import concourse.bass as bass
    import concourse.bacc as bacc
    import concourse.mybir as mybir
    import concourse.tile as tile
    from concourse.masks import make_identity

    c = cfg
    RT, RU_PAD, RK_PAD = meta["RT"], meta["RU_PAD"], meta["RK_PAD"]
    MP = meta["MP"]
    f32, bf, i16 = mybir.dt.float32, mybir.dt.bfloat16, mybir.dt.int16
    MUL, ADD = mybir.AluOpType.mult, mybir.AluOpType.add
    SUB = mybir.AluOpType.subtract

    nc = bacc.Bacc("TRN2", target_bir_lowering=False, debug=False,
                   num_devices=c.C)

    def din(name, shape, dt):
        return nc.dram_tensor(name, shape, dt, kind="ExternalInput")

    xT = din("xT", [512, RT], bf)
    dloc = [din("dloc_td", [128, RT // 128], f32), din("dloc_bu", [128, RT // 128], f32)]
    ddst = [din("ddst_td", [128, c.NB], f32), din("ddst_bu", [128, c.NB], f32)]
    s1 = [din("s_td1", [128, meta["sched1"][0]["TOTKT"] * 128], bf),
          din("s_bu1", [128, meta["sched1"][1]["TOTKT"] * 128], bf)]
    i1 = [din("i_td1", [128, meta["sched1"][0]["TOTKT"] * 8], i16),
          din("i_bu1", [128, meta["sched1"][1]["TOTKT"] * 8], i16)]
    s2 = [din("s_td2", [128, meta["sched2"][0]["TOTKT"] * 128], bf),
          din("s_bu2", [128, meta["sched2"][1]["TOTKT"] * 128], bf)]
    i2 = [din("i_td2", [128, meta["sched2"][0]["TOTKT"] * 8], i16),
          din("i_bu2", [128, meta["sched2"][1]["TOTKT"] * 8], i16)]
    midx = din("midx", [128, MP // 16], i16)
    mw_t = din("mw", [128, MP // 128], f32)
    ppool_t = din("ppool", [128, c.NB * 128], f32)
    w1all = din("w1all", [512, 512], bf)
    w2 = [din("w2_td", [128, 128], bf), din("w2_bu", [128, 128], bf)]
    tonbc = din("tonbc", [128, 256], bf)
    b1bc = [din("b1bc_td", [128, 256], f32), din("b1bc_bu", [128, 256], f32)]
    b2bc = [din("b2bc_td", [128, 128], f32), din("b2bc_bu", [128, 128], f32)]
    ones_t = din("ones", [128, 1], f32)
    gmask_t = din("gmask", [128, 1], f32)
    loss_t = nc.dram_tensor("loss", [1, 1], f32, kind="ExternalOutput")

    z_t = nc.dram_tensor("zarr", [RT, 512], bf, kind="Internal")

    with tile.TileContext(nc) as tc:
        with (
            tc.tile_pool(name="const", bufs=1) as cpool,
            tc.tile_pool(name="dram", bufs=1, space="DRAM") as dpool,
        ):
            z2own = [dpool.tile([c.OWNP, 128], bf, tag=f"z2own{d}", name=f"z2own{d}") for d in range(2)]
            z2full = [dpool.tile([c.NPAD, 128], bf, addr_space="Shared", tag=f"z2full{d}", name=f"z2full{d}")
                      for d in range(2)]
            hown = [dpool.tile([c.OWNP, 128], f32, tag=f"hown{d}", name=f"hown{d}") for d in range(2)]
            ar_in = dpool.tile([128, 260], f32, tag="arin", name="arin")
            ar_out = dpool.tile([128, 260], f32, addr_space="Shared", tag="arout", name="arout")

            ident = cpool.tile([128, 128], bf)
            make_identity(nc, ident[:])
            w1sb = cpool.tile([128, 4 * 512], bf)
            for k in range(4):
                nc.sync.dma_start(out=w1sb[:, k * 512:(k + 1) * 512],
                                  in_=w1all[k * 128:(k + 1) * 128, :])
            w2sb = [cpool.tile([128, 128], bf, tag=f"w2_{d}", name=f"w2_{d}") for d in range(2)]
            tonsb = cpool.tile([128, 256], bf)
            b1sb = [cpool.tile([128, 256], f32, tag=f"b1_{d}", name=f"b1_{d}") for d in range(2)]
            b2sb = [cpool.tile([128, 128], f32, tag=f"b2_{d}", name=f"b2_{d}") for d in range(2)]
            dlsb = [cpool.tile([128, RT // 128], f32, tag=f"dl_{d}", name=f"dl_{d}") for d in range(2)]
            ddsb = [cpool.tile([128, c.NB], f32, tag=f"dd_{d}", name=f"dd_{d}") for d in range(2)]
            onesb = cpool.tile([128, 1], f32)
            gmsb = cpool.tile([128, 1], f32)
            nc.sync.dma_start(out=tonsb[:], in_=tonbc[:, :])
            nc.sync.dma_start(out=onesb[:], in_=ones_t[:, :])
            nc.sync.dma_start(out=gmsb[:], in_=gmask_t[:, :])
            for d in range(2):
                nc.sync.dma_start(out=w2sb[d][:], in_=w2[d][:, :])
                nc.sync.dma_start(out=b1sb[d][:], in_=b1bc[d][:, :])
                nc.sync.dma_start(out=b2sb[d][:], in_=b2bc[d][:, :])
                nc.sync.dma_start(out=dlsb[d][:], in_=dloc[d][:, :])
                nc.sync.dma_start(out=ddsb[d][:], in_=ddst[d][:, :])

            # ================= P1: z = scaled([x1|x] @ W1-fused) ==========
            with (
                tc.tile_pool(name="xk", bufs=2) as xkp,
                tc.tile_pool(name="zsb", bufs=3) as zsp,
                tc.tile_pool(name="pz", bufs=2, space="PSUM") as pzp,
            ):
                for sec, (r0, rlen) in enumerate(((0, RU_PAD), (RU_PAD, RK_PAD))):
                    for off in range(0, rlen, c.NF):
                        nf = min(c.NF, rlen - off)
                        xk = xkp.tile([128, 4 * c.NF], bf, tag="xk", name="xk")
                        for k in range(4):
                            nc.sync.dma_start(
                                out=xk[:, k * c.NF:k * c.NF + nf],
                                in_=xT[k * 128:(k + 1) * 128, r0 + off:r0 + off + nf])
                        for j in range(nf // 128):
                            row = r0 + off + j * 128
                            rb = row // 128
                            if sec == 0:
                                ps = pzp.tile([128, 512], f32, tag="pz", name="pz")
                                for k in range(4):
                                    nc.tensor.matmul(
                                        out=ps[:],
                                        lhsT=xk[:, k * c.NF + j * 128:k * c.NF + (j + 1) * 128],
                                        rhs=w1sb[:, k * 512:(k + 1) * 512],
                                        start=(k == 0), stop=(k == 3))
                                zs = zsp.tile([128, 512], bf, tag="zs", name="zs")
                                nc.scalar.activation(
                                    out=zs[:, 0:256], in_=ps[:, 0:256],
                                    func=mybir.ActivationFunctionType.Copy,
                                    scale=dlsb[0][:, rb:rb + 1])
                                nc.scalar.activation(
                                    out=zs[:, 256:512], in_=ps[:, 256:512],
                                    func=mybir.ActivationFunctionType.Copy,
                                    scale=dlsb[1][:, rb:rb + 1])
                            else:
                                ps = pzp.tile([128, 512], f32, tag="pz", name="pz")
                                for k in range(4):
                                    nc.tensor.matmul(
                                        out=ps[:, 0:128],
                                        lhsT=xk[:, k * c.NF + j * 128:k * c.NF + (j + 1) * 128],
                                        rhs=w1sb[:, k * 512 + 128:k * 512 + 256],
                                        start=(k == 0), stop=(k == 3))
                                for k in range(4):
                                    nc.tensor.matmul(
                                        out=ps[:, 128:256],
                                        lhsT=xk[:, k * c.NF + j * 128:k * c.NF + (j + 1) * 128],
                                        rhs=w1sb[:, k * 512 + 384:k * 512 + 512],
                                        start=(k == 0), stop=(k == 3))
                                zs = zsp.tile([128, 512], bf, tag="zs", name="zs")
                                nc.vector.tensor_scalar(
                                    out=zs[:, 0:128], in0=tonsb[:, 0:128],
                                    scalar1=dlsb[0][:, rb:rb + 1], scalar2=None, op0=MUL)
                                nc.scalar.activation(
                                    out=zs[:, 128:256], in_=ps[:, 0:128],
                                    func=mybir.ActivationFunctionType.Copy,
                                    scale=dlsb[0][:, rb:rb + 1])
                                nc.vector.tensor_scalar(
                                    out=zs[:, 256:384], in0=tonsb[:, 128:256],
                                    scalar1=dlsb[1][:, rb:rb + 1], scalar2=None, op0=MUL)
                                nc.scalar.activation(
                                    out=zs[:, 384:512], in_=ps[:, 128:256],
                                    func=mybir.ActivationFunctionType.Copy,
                                    scale=dlsb[1][:, rb:rb + 1])
                            nc.sync.dma_start(out=z_t[row:row + 128, :], in_=zs[:])

            # ================= helper: one aggregation layer ==============
            def agg_layer(layer, d, sch, s_in, i_in, src_t, src_cols, elem,
                          estep, poolps):
                NW, KT, ktoff = sch["NW"], sch["KT"], sch["ktoff"]
                wlen = lambda w: min(c.WIN, (RT if layer == 0 else c.NPAD) - w * c.WIN)
                with (
                    tc.tile_pool(name=f"g{layer}{d}", bufs=2) as gp,
                    tc.tile_pool(name=f"sI{layer}{d}", bufs=2) as sp,
                    tc.tile_pool(name=f"ix{layer}{d}", bufs=2) as ip,
                    tc.tile_pool(name=f"fin{layer}{d}", bufs=3) as fp,
                    tc.tile_pool(name=f"pp{layer}{d}", bufs=2) as ppp,
                    tc.tile_pool(name=f"agg{layer}{d}", bufs=2, space="PSUM") as ap,
                    tc.tile_pool(name=f"tr{layer}{d}", bufs=2, space="PSUM") as trp,
                ):
                    GB = sch["GB"]
                    for g, ops in enumerate(sch["groups"]):
                        blks = range(g * GB, min((g + 1) * GB, c.NB))
                        if not ops:
                            continue
                        gkt0 = ops[0][1]
                        gnkt = sum(nk for _, _, nk in ops)
                        st = sp.tile([128, gnkt * 128], bf, tag="s", name="s")
                        nc.sync.dma_start(
                            out=st[:], in_=s_in[:, gkt0 * 128:(gkt0 + gnkt) * 128])
                        it = ip.tile([128, gnkt * 8], i16, tag="i", name="i")
                        nc.sync.dma_start(
                            out=it[:], in_=i_in[:, gkt0 * 8:(gkt0 + gnkt) * 8])
                        gt = gp.tile([128, gnkt * elem], bf, tag="g", name="g")
                        optiles = {}
                        for w, base, nk in ops:
                            o = base - gkt0
                            nc.gpsimd.dma_gather(
                                gt[:, o * elem:(o + nk) * elem].rearrange(
                                    "p (k e) -> p k e", k=nk, e=elem),
                                src_t[w * c.WIN:w * c.WIN + wlen(w),
                                      src_cols[0]:src_cols[1]],
                                it[:, o * 8:(o + nk) * 8], nk * 128, nk * 128, elem,
                                elem_step=estep, single_packet=False)
                            optiles[w] = (gt, gkt0)
                        if layer == 0:
                            ptile = None
                        else:
                            ptile = ppp.tile([128, len(blks) * 128], f32, tag="pp", name="pp")
                            b0 = g * GB
                            nc.sync.dma_start(
                                out=ptile[:],
                                in_=ppool_t[:, b0 * 128:(b0 + len(blks)) * 128])
                        for b in blks:
                            nkb = int(KT[b].sum())
                            if nkb == 0:
                                continue
                            fw = 256 if layer == 0 else 128
                            ps = ap.tile([128, fw], f32, tag="a", name="a")
                            emitted = 0
                            for w in range(NW):
                                if KT[b, w] == 0:
                                    continue
                                gt, base = optiles[w]
                                for t in range(KT[b, w]):
                                    kt = ktoff[b, w] + t
                                    nc.tensor.matmul(
                                        out=ps[:],
                                        lhsT=st[:, (kt - gkt0) * 128:(kt - gkt0 + 1) * 128],
                                        rhs=gt[:, (kt - base) * elem:(kt - base + 1) * elem],
                                        start=(emitted == 0),
                                        stop=(emitted == nkb - 1))
                                    emitted += 1
                            # finalize: (ps * dinv_dst) + bias
                            bias = b1sb[d] if layer == 0 else b2sb[d]
                            nc.vector.scalar_tensor_tensor(
                                out=ps[:], in0=ps[:], scalar=ddsb[d][:, b:b + 1],
                                in1=bias[:, 0:fw], op0=MUL, op1=ADD)
                            if layer == 0:
                                h1 = fp.tile([128, 256], bf, tag="h1", name="h1")
                                nc.scalar.activation(
                                    out=h1[:], in_=ps[:],
                                    func=mybir.ActivationFunctionType.Relu)
                                trt = trp.tile([128, 256], bf, tag="t", name="t")
                                nc.tensor.transpose(
                                    out=trt[:, 0:128], in_=h1[:, 0:128], identity=ident[:])
                                nc.tensor.transpose(
                                    out=trt[:, 128:256], in_=h1[:, 128:256], identity=ident[:])
                                h1T = fp.tile([128, 256], bf, tag="h1T", name="h1T")
                                nc.vector.tensor_copy(out=h1T[:], in_=trt[:])
                                z2ps = trp.tile([128, 128], f32, tag="z2", name="z2")
                                nc.tensor.matmul(out=z2ps[:, 0:64],
                                                 lhsT=h1T[:, 0:128],
                                                 rhs=w2sb[d][:, 0:64],
                                                 start=True, stop=True)
                                nc.tensor.matmul(out=z2ps[:, 64:128],
                                                 lhsT=h1T[:, 128:256],
                                                 rhs=w2sb[d][:, 64:128],
                                                 start=True, stop=True)
                                z2sb = fp.tile([128, 128], bf, tag="z2sb", name="z2sb")
                                nc.scalar.activation(
                                    out=z2sb[:], in_=z2ps[:],
                                    func=mybir.ActivationFunctionType.Copy,
                                    scale=ddsb[d][:, b:b + 1])
                                nc.sync.dma_start(
                                    out=z2own[d][b * 128:(b + 1) * 128, :], in_=z2sb[:])
                            else:
                                hsb = fp.tile([128, 128], f32, tag="hsb", name="hsb")
                                nc.scalar.copy(out=hsb[:], in_=ps[:])
                                nc.tensor.matmul(
                                    out=poolps[:], lhsT=ptile[:, (b - g * GB) * 128:(b - g * GB + 1) * 128],
                                    rhs=hsb[:], start=(b == 0), stop=(b == c.NB - 1),
                                    skip_group_check=True)
                                nc.sync.dma_start(
                                    out=hown[d][b * 128:(b + 1) * 128, :], in_=hsb[:])

            # ================= L1 (both dirs) =============================
            for d in range(2):
                agg_layer(0, d, meta["sched1"][d], s1[d], i1[d],
                          z_t, (256 * d, 256 * d + 256), 256, 512, None)

            # ================= AllGather z2 ===============================
            for d in range(2):
                nc.gpsimd.collective_compute(
                    "AllGather", mybir.AluOpType.bypass,
                    replica_groups=[list(range(c.C))],
                    ins=[z2own[d].opt()], outs=[z2full[d].opt()])

            # ================= L2 (both dirs) =============================
            with tc.tile_pool(name="plps", bufs=2, space="PSUM") as plp:
                poolps = [plp.tile([128, 128], f32, tag=f"pl{d}", name=f"pl{d}") for d in range(2)]
                for d in range(2):
                    agg_layer(1, d, meta["sched2"][d], s2[d], i2[d],
                              z2full[d], (0, 128), 128, None, poolps[d])

                # ============= masked-node cosine partial ================
                def cos_terms(sp_, a1, a2, b1_, b2_, tag):
                    """-> (dot, n1, n2) [128,1] f32 tiles"""
                    outs = []
                    for (u, v) in ((a1, a2), (a1, a1), (a2, a2)):
                        acc1 = sp_.tile([128, 1], f32, tag=f"{tag}ac1", name=f"{tag}ac1")
                        acc2 = sp_.tile([128, 1], f32, tag=f"{tag}ac2", name=f"{tag}ac2")
                        scr = sp_.tile([128, 64], f32, tag=f"{tag}scr", name=f"{tag}scr")
                        nc.vector.scalar_tensor_tensor(
                            out=scr[:], in0=u[0], scalar=1.0, in1=v[0],
                            op0=MUL, op1=MUL, accum_out=acc1[:])
                        nc.vector.scalar_tensor_tensor(
                            out=scr[:], in0=u[1], scalar=1.0, in1=v[1],
                            op0=MUL, op1=MUL, accum_out=acc2[:])
                        s = sp_.tile([128, 1], f32, tag=f"{tag}s", name=f"{tag}s")
                        nc.vector.tensor_tensor(out=s[:], in0=acc1[:], in1=acc2[:], op=ADD)
                        outs.append(s)
                    return outs

                def rcp_guard(sp_, n, tag):
                    r = sp_.tile([128, 1], f32, tag=f"{tag}r", name=f"{tag}r")
                    nc.scalar.sqrt(out=r[:], in_=n[:])
                    nc.vector.tensor_scalar_max(out=r[:], in0=r[:], scalar1=1e-12)
                    nc.vector.reciprocal(out=r[:], in_=r[:])
                    return r

                with (
                    tc.tile_pool(name="msk", bufs=2) as mp_,
                    tc.tile_pool(name="msc", bufs=4) as sc_,
                    tc.tile_pool(name="scps", bufs=2, space="PSUM") as scp,
                ):
                    macc = cpool.tile([128, 1], f32)
                    nc.vector.memset(macc[:], 0.0)
                    MOPS = MP // 2048
                    for o in range(MOPS):
                        it = mp_.tile([128, 128], i16, tag="mi", name="mi")
                        nc.sync.dma_start(out=it[:], in_=midx[:, o * 128:(o + 1) * 128])
                        wt = mp_.tile([128, 16], f32, tag="mwt", name="mwt")
                        nc.sync.dma_start(out=wt[:], in_=mw_t[:, o * 16:(o + 1) * 16])
                        gts = []
                        for d in range(2):
                            gt = mp_.tile([128, 16 * 128], f32, tag=f"mg{d}", name=f"mg{d}")
                            nc.gpsimd.dma_gather(
                                gt[:].rearrange("p (k e) -> p k e", k=16, e=128),
                                hown[d][:, :], it[:], 2048, 2048, 128,
                                elem_step=None, single_packet=False)
                            gts.append(gt)
                        for k in range(16):
                            atd = gts[0][:, k * 128:(k + 1) * 128]
                            abu = gts[1][:, k * 128:(k + 1) * 128]
                            dot, n1, n2 = cos_terms(
                                sc_, (atd[:, 0:64], abu[:, 0:64]),
                                (atd[:, 64:128], abu[:, 64:128]),
                                None, None, "m")
                            r1 = rcp_guard(sc_, n1, "m1")
                            r2 = rcp_guard(sc_, n2, "m2")
                            cosv = sc_.tile([128, 1], f32, tag="mcos", name="mcos")
                            nc.vector.tensor_tensor(out=cosv[:], in0=dot[:], in1=r1[:], op=MUL)
                            nc.vector.tensor_tensor(out=cosv[:], in0=cosv[:], in1=r2[:], op=MUL)
                            u = sc_.tile([128, 1], f32, tag="mu", name="mu")
                            nc.vector.tensor_tensor(out=u[:], in0=cosv[:],
                                                    in1=wt[:, k:k + 1], op=MUL)
                            term = sc_.tile([128, 1], f32, tag="mt", name="mt")
                            nc.vector.tensor_tensor(out=term[:], in0=wt[:, k:k + 1],
                                                    in1=u[:], op=SUB)
                            nc.vector.tensor_tensor(out=macc[:], in0=macc[:],
                                                    in1=term[:], op=ADD)
                    msps = scp.tile([1, 1], f32, tag="ms", name="ms")
                    nc.tensor.matmul(out=msps[:], lhsT=macc[:], rhs=onesb[:],
                                     start=True, stop=True)

                    # ============= pool partials -> AllReduce ============
                    arsb = cpool.tile([128, 260], f32)
                    nc.vector.memset(arsb[:], 0.0)
                    nc.vector.tensor_copy(out=arsb[:, 0:128], in_=poolps[0][:])
                    nc.vector.tensor_copy(out=arsb[:, 128:256], in_=poolps[1][:])
                    nc.vector.tensor_copy(out=arsb[0:1, 256:257], in_=msps[:])
                    nc.sync.dma_start(out=ar_in[:, :], in_=arsb[:])
                    nc.gpsimd.collective_compute(
                        "AllReduce", mybir.AluOpType.add,
                        replica_groups=[list(range(c.C))],
                        ins=[ar_in.opt()], outs=[ar_out.opt()])
                    ar2 = cpool.tile([128, 260], f32)
                    nc.sync.dma_start(out=ar2[:], in_=ar_out[:, :])

                    # ============= pooled cosine + final loss ============
                    dot, n1, n2 = cos_terms(
                        sc_, (ar2[:, 0:64], ar2[:, 128:192]),
                        (ar2[:, 64:128], ar2[:, 192:256]), None, None, "g")
                    r1 = rcp_guard(sc_, n1, "g1")
                    r2 = rcp_guard(sc_, n2, "g2")
                    cosg = sc_.tile([128, 1], f32, tag="gcos", name="gcos")
                    nc.vector.tensor_tensor(out=cosg[:], in0=dot[:], in1=r1[:], op=MUL)
                    nc.vector.tensor_tensor(out=cosg[:], in0=cosg[:], in1=r2[:], op=MUL)
                    gterm = sc_.tile([128, 1], f32, tag="gt", name="gt")
                    nc.vector.tensor_scalar(out=gterm[:], in0=cosg[:],
                                            scalar1=-1.0, scalar2=1.0,
                                            op0=MUL, op1=ADD)
                    nc.vector.tensor_tensor(out=gterm[:], in0=gterm[:],
                                            in1=gmsb[:], op=MUL)
                    gsps = scp.tile([1, 1], f32, tag="gs", name="gs")
                    nc.tensor.matmul(out=gsps[:], lhsT=gterm[:], rhs=onesb[:],
                                     start=True, stop=True)
                    l1t = sc_.tile([1, 1], f32, tag="l1", name="l1")
                    nc.scalar.activation(out=l1t[:], in_=gsps[:],
                                         func=mybir.ActivationFunctionType.Copy,
                                         scale=1.0 / c.G)
                    l2t = sc_.tile([1, 1], f32, tag="l2", name="l2")
                    nc.scalar.activation(out=l2t[:], in_=ar2[0:1, 256:257],
                                         func=mybir.ActivationFunctionType.Copy,
                                         scale=1.0 / c.M)
                    nc.vector.tensor_tensor(out=l1t[:], in0=l1t[:], in1=l2t[:], op=ADD)
                    nc.sync.dma_start(out=loss_t[:, :], in_=l1t[:])

    return nc


# ---------------------------------------------------------------- entry

LAST_RESULT = None


def kernel(_trace=False, **inputs):
    global LAST_RESULT
    import time
    from concourse import bass_utils
    cfg = FULL
    t0 = time.monotonic()
    meta, in_maps = host_prep(cfg, inputs)
    t1 = time.monotonic()
    nc = build_program(cfg, meta)
    t2 = time.monotonic()
    nc.compile()
    t3 = time.monotonic()
    res = bass_utils.run_bass_kernel_spmd(
        nc, in_maps, core_ids=list(range(cfg.C)),
        trace=_trace, trace_cores=[0] if _trace else None)
    t4 = time.monotonic()
    print(f"[kernel] prep {t1-t0:.1f}s build {t2-t1:.1f}s "
          f"compile {t3-t2:.1f}s run {t4-t3:.1f}s", file=sys.stderr)
    LAST_RESULT = res
    return np.float32(res.results[0]["loss"][0, 0])



# revision 5
# speedup vs baseline: 2.1150x; 2.1150x over previous
"""Trainium2 Bass kernel for the rumor-GCN masked-autoencoder loss (v3).

Strategy (8 NeuronCores, SPMD single NEFF):
  - Layer 1 message passing is HOST-PREGATHERED: for each core/direction the
    host materializes xe[slot] = x[src[slot]] in (dst-block, k-tile) slot
    order, shipped pre-tiled [128, KT*512] bf16.  The kernel streams it with
    large sequential DMAs -- zero gather descriptors for layer 1.
  - GCN norms (dinv[src]*dinv[dst], self loop dinv^2) are folded into the
    one-hot S matrices, generated ON-CHIP per k-tile from f32 metadata
    columns (dst lane, val_tgt, val_on) via iota-is_equal * val on GpSimd.
    The "on" encoder's masked input x1 is handled by val_on = val * (src not
    masked); a nonzero mask token adds host-built token slots.
  - Aggregation in AT-form (psA[kchunk, node]); W1/W2 applied per 4-block
    group with wide 512-col matmuls in feature-major form; z2 transposed
    back per block for the AllGather and stashed in SBUF.
  - Layer 2: z2 AllGathered (bf16), dma_gather per (8-block supergroup, 32K
    window) with (4-block group, window) k-tile buckets; self-loop terms
    come from the SBUF z2 stash via one diag-matmul per block (no gather
    slots for self loops).
  - Pooling via on-chip batch-onehot matmuls into persistent PSUM; the
    masked-node cosine loss is computed per block inside the L2-BU loop
    (wide-tile tail), one small AllReduce finishes the scalar loss.
"""

import sys

import numpy as np

sys.path.insert(0, "/opt/trn_rl_repo")

# ---------------------------------------------------------------- config


class Cfg:
    def __init__(self, N, E, G, M, C=8, WIN=32768, GB1=4, SGF=2):
        self.N, self.E, self.G, self.M, self.C = N, E, G, M, C
        self.IN, self.HID, self.OUT = 512, 128, 64
        self.WIN = WIN
        self.GB1 = GB1          # blocks per psum group (L1 and L2)
        self.SGF = SGF          # L2: psum groups per gather supergroup
        assert N % C == 0
        self.OWN = N // C
        self.NB = -(-self.OWN // 128)
        self.OWNP = self.NB * 128
        self.NPAD = C * self.OWNP
        self.NW2 = -(-self.NPAD // WIN)
        self.NG = -(-self.NB // GB1)
        self.NSG = -(-self.NG // SGF)


FULL = Cfg(N=200000, E=400000, G=128, M=100000)

_WNAMES = [p + s for p in ("on_td", "on_bu", "tgt_td", "tgt_bu")
           for s in ("_W1", "_b1", "_W2", "_b2")]


def _rep16(idx_flat):
    """int16 index list -> [128, n//16] layout (16-part wrap, 8x replicated)."""
    n = len(idx_flat)
    assert n % 16 == 0
    blk = np.zeros((16, n // 16), dtype=np.int16)
    k = np.arange(n)
    blk[k % 16, k // 16] = idx_flat
    return np.tile(blk, (8, 1))


def _bcast(vec, parts=128):
    return np.broadcast_to(np.asarray(vec, np.float32)[None, :],
                           (parts, len(vec))).copy()


def _tile_rows(a, ncols):
    """[KT*128, ncols] row-major -> [128, KT*ncols] (slot k*128+p at
    partition p, col-block k)."""
    kt = a.shape[0] // 128
    return np.ascontiguousarray(
        a.reshape(kt, 128, ncols).transpose(1, 0, 2).reshape(128, kt * ncols))


# ---------------------------------------------------------------- host prep


def _assign_nodes(cfg, wtd, wbu):
    """Greedy degree-balanced node -> (core, block, lane) assignment."""
    c = cfg
    w = wtd + wbu
    order = np.argsort(-w, kind="stable")
    core = np.empty(c.N, np.int64)
    grid = np.arange(c.N) % (2 * c.C)
    snake = np.where(grid < c.C, grid, 2 * c.C - 1 - grid)
    core[order] = snake
    blk = np.empty(c.N, np.int64)
    lane = np.empty(c.N, np.int64)
    for ci in range(c.C):
        nodes = order[core[order] == ci]          # sorted by weight desc
        nb = c.NB
        g = np.arange(len(nodes)) % (2 * nb)
        snb = np.where(g < nb, g, 2 * nb - 1 - g)
        blk[nodes] = snb
        ord2 = np.argsort(snb, kind="stable")
        srt = snb[ord2]
        start = np.r_[True, srt[1:] != srt[:-1]]
        segstart = np.maximum.accumulate(
            np.where(start, np.arange(len(srt)), 0))
        lane[nodes[ord2]] = np.arange(len(srt)) - segstart
    assert lane.max() < 128
    return core, blk, lane


def host_prep(cfg, inp):
    import ml_dtypes
    bf16 = ml_dtypes.bfloat16
    c = cfg
    x = np.asarray(inp["x"], np.float32)
    token = np.asarray(inp["enc_mask_token"], np.float32).reshape(-1)
    token_zero = not np.any(token)
    tokbf = token.astype(bf16)
    ei = np.asarray(inp["edge_index"])
    src, dst = ei[0].astype(np.int64), ei[1].astype(np.int64)
    batch = np.asarray(inp["batch"]).astype(np.int64)
    mask_nodes = np.asarray(inp["mask_nodes"]).astype(np.int64)
    W = {k: np.asarray(inp[k], np.float32) for k in _WNAMES}

    wtd = np.bincount(dst, minlength=c.N).astype(np.int64)
    wbu = np.bincount(src, minlength=c.N).astype(np.int64)
    dinv = [(1.0 / np.sqrt(wtd + 1.0)).astype(np.float32),
            (1.0 / np.sqrt(wbu + 1.0)).astype(np.float32)]
    mcount = np.bincount(mask_nodes, minlength=c.N).astype(np.float32)
    is_masked = mcount > 0
    xbf = x.astype(bf16)

    core, blk, lane = _assign_nodes(c, wtd, wbu)
    z2row = core * c.OWNP + blk * 128 + lane

    ed = {}
    for d in range(2):
        ad, asr = (dst, src) if d == 0 else (src, dst)
        val = dinv[d][ad] * dinv[d][asr]
        ed[d] = (ad, asr, val)

    # ---- per-(core,dir,block) L1 slot counts (edges + self + token slots)
    all_nodes = np.arange(c.N, dtype=np.int64)
    e_full = {}   # (core,dir) -> (dlane, dblk, src(-1=token), val_tgt, val_on)
    for ci in range(c.C):
        own = core == ci
        on_nodes = all_nodes[own]
        for d in range(2):
            ad, asr, val = ed[d]
            sel = core[ad] == ci
            dl = np.concatenate([lane[ad[sel]], lane[on_nodes]])
            db = np.concatenate([blk[ad[sel]], blk[on_nodes]])
            sr = np.concatenate([asr[sel], on_nodes])
            vv = np.concatenate([val[sel], dinv[d][on_nodes] ** 2])
            vt = vv
            vo = vv * (~is_masked[sr])
            if not token_zero:
                msk = is_masked[sr]
                cacc = np.zeros((c.NB, 128), np.float32)
                np.add.at(cacc, (db[msk], dl[msk]), vv[msk])
                nz = np.nonzero(cacc)
                dl = np.concatenate([dl, nz[1]])
                db = np.concatenate([db, nz[0]])
                sr = np.concatenate([sr, np.full(len(nz[0]), -1, np.int64)])
                vt = np.concatenate([vt, np.zeros(len(nz[0]), np.float32)])
                vo = np.concatenate([vo, cacc[nz]])
            e_full[ci, d] = (dl, db, sr, vt, vo)

    cnt1 = np.zeros((2, c.C, c.NB), np.int64)
    for (ci, d), (dl, db, sr, vt, vo) in e_full.items():
        np.add.at(cnt1[d, ci], db, 1)
    KT1 = [np.maximum(1, -(-cnt1[d].max(axis=0) // 128)) for d in range(2)]
    KT1off = [np.r_[0, np.cumsum(KT1[d])].astype(np.int64) for d in range(2)]
    TOTKT1 = [int(KT1off[d][-1]) for d in range(2)]

    # ---- L2 schedule: buckets (g4, w)
    cnt2 = np.zeros((2, c.C, c.NG, c.NW2), np.int64)
    for d in range(2):
        ad, asr, _ = ed[d]
        np.add.at(cnt2[d], (core[ad], blk[ad] // c.GB1, z2row[asr] // c.WIN), 1)
    KT2 = [-(-cnt2[d].max(axis=0) // 128) for d in range(2)]   # [NG, NW2]
    KT2off, sched2, TOTKT2 = [], [], []
    for d in range(2):
        off = np.zeros((c.NG, c.NW2), np.int64)
        acc = 0
        sgs = []
        for sg in range(c.NSG):
            g4s = range(sg * c.SGF, min((sg + 1) * c.SGF, c.NG))
            ops = []
            for w in range(c.NW2):
                nk = int(sum(KT2[d][g, w] for g in g4s))
                if nk == 0:
                    continue
                base = acc
                for g in g4s:
                    off[g, w] = acc
                    acc += KT2[d][g, w]
                ops.append((w, base, nk))
            sgs.append(ops)
        KT2off.append(off)
        sched2.append(sgs)
        TOTKT2.append(int(acc))

    # ---- shared (replicated) tensors
    iota512 = _bcast(np.arange(512))
    ident = np.eye(128, dtype=np.float32)
    w1 = {}
    w2 = {}
    b1c = {}
    b2col = {}
    b1bc = {}
    for d, nm in ((0, "td"), (1, "bu")):
        for v, pre in ((0, "on"), (1, "tgt")):
            wt = W[f"{pre}_{nm}_W1"]                     # [512, 128]
            w1[d, v] = np.ascontiguousarray(
                wt.reshape(4, 128, 128).transpose(1, 0, 2).reshape(128, 512)
            ).astype(bf16)
            w2[d, v] = W[f"{pre}_{nm}_W2"].astype(bf16)  # [128, 64]
        b1c[d] = np.stack([W[f"on_{nm}_b1"], W[f"tgt_{nm}_b1"]],
                          axis=1).astype(np.float32)     # [128, 2]
        b2col[d] = np.concatenate(
            [W[f"on_{nm}_b2"], W[f"tgt_{nm}_b2"]]).astype(np.float32)[:, None]
    ones = np.ones((128, 1), np.float32)
    gmask = np.zeros((128, 1), np.float32)
    gmask[:c.G, 0] = 1.0

    # ---- per-core inputs
    in_maps = []
    for ci in range(c.C):
        own_sel = core == ci
        m = {}

        def nodecol(valarr, pad=0.0):
            a = np.full((128, c.NB), pad, np.float32)
            a[lane[own_sel], blk[own_sel]] = valarr[own_sel]
            return a

        m["batchcol"] = nodecol(batch.astype(np.float32), pad=-1.0)
        m["mw"] = nodecol(mcount)
        for d, nm in ((0, "td"), (1, "bu")):
            m[f"sf_{nm}"] = nodecol(dinv[d] * dinv[d])

        for d, nm in ((0, "td"), (1, "bu")):
            dl, db, sr, vt, vo = e_full[ci, d]
            # --- L1 slots ---
            order = np.argsort(db, kind="stable")
            sdb, sdl = db[order], dl[order]
            ssrc, svt, svo = sr[order], vt[order], vo[order]
            segchange = np.r_[True, sdb[1:] != sdb[:-1]]
            segstart = np.maximum.accumulate(
                np.where(segchange, np.arange(len(sdb)), 0))
            pos = np.arange(len(sdb)) - segstart
            slot = KT1off[d][sdb] * 128 + pos
            nslot1 = TOTKT1[d] * 128
            assert pos.max(initial=0) < KT1[d][sdb].max() * 128 or True
            assert slot.max(initial=0) < nslot1
            xe = np.zeros((nslot1, 512), bf16)
            reg = ssrc >= 0
            xe[slot[reg]] = xbf[ssrc[reg]]
            if not token_zero:
                xe[slot[~reg]] = tokbf
            m[f"xe_{nm}"] = _tile_rows(xe, 512)
            m1 = np.zeros((nslot1, 4), np.float32)
            m1[:, 0] = 1000.0
            m1[slot, 0] = sdl
            m1[slot, 1] = svt
            m1[slot, 2] = svo
            m[f"m1_{nm}"] = _tile_rows(m1, 4)

            # --- L2 slots (edges only) ---
            ad, asr, val = ed[d]
            sel = core[ad] == ci
            f_db = blk[ad[sel]]
            f_g4 = f_db // c.GB1
            f_col = (f_db % c.GB1) * 128 + lane[ad[sel]]
            f_row = z2row[asr[sel]]
            f_w = f_row // c.WIN
            f_rel = f_row - f_w * c.WIN
            f_val = val[sel]
            # order must match device slot order: sg-major, w, g4
            sgid = f_g4 // c.SGF
            order = np.lexsort((np.arange(len(f_g4)), f_g4, f_w, sgid))
            key = f_g4[order] * c.NW2 + f_w[order]
            segchange = np.r_[True, key[1:] != key[:-1]]
            segstart = np.maximum.accumulate(
                np.where(segchange, np.arange(len(key)), 0))
            pos = np.arange(len(key)) - segstart
            slot = KT2off[d][f_g4[order], f_w[order]] * 128 + pos
            nslot2 = TOTKT2[d] * 128
            assert slot.max(initial=0) < nslot2
            idx = np.zeros(nslot2, np.int64)
            idx[slot] = f_rel[order]
            m[f"i_{nm}"] = _rep16(idx.astype(np.int16))
            m2 = np.zeros((nslot2, 2), np.float32)
            m2[:, 0] = 1000.0
            m2[slot, 0] = f_col[order]
            m2[slot, 1] = f_val[order]
            m[f"m2_{nm}"] = _tile_rows(m2, 2)

        m.update(iota512=iota512, ident=ident, ones=ones, gmask=gmask)
        for d, nm in ((0, "td"), (1, "bu")):
            m[f"w1on_{nm}"] = w1[d, 0]
            m[f"w1tg_{nm}"] = w1[d, 1]
            m[f"w2on_{nm}"] = w2[d, 0]
            m[f"w2tg_{nm}"] = w2[d, 1]
            m[f"b1c_{nm}"] = b1c[d]
            m[f"b2_{nm}"] = b2col[d]
        in_maps.append(m)

    meta = dict(KT1=KT1, TOTKT1=TOTKT1, KT2=KT2, KT2off=KT2off,
                sched2=sched2, TOTKT2=TOTKT2,
                assign=(core, blk, lane), z2row=z2row)
    return meta, in_maps


# ---------------------------------------------------------------- program


def build_program(cfg, meta):
    import concourse.bacc as bacc
    import concourse.mybir as mybir
    import concourse.tile as tile

    c = cfg
    KT1, TOTKT1 = meta["KT1"], meta["TOTKT1"]
    KT2, KT2off, TOTKT2 = meta["KT2"], meta["KT2off"], meta["TOTKT2"]
    sched2 = meta["sched2"]
    f32, bf, i16 = mybir.dt.float32, mybir.dt.bfloat16, mybir.dt.int16
    MUL, ADD, SUB = (mybir.AluOpType.mult, mybir.AluOpType.add,
                     mybir.AluOpType.subtract)
    EQ = mybir.AluOpType.is_equal
    AF = mybir.ActivationFunctionType

    nc = bacc.Bacc("TRN2", target_bir_lowering=False, debug=False,
                   num_devices=c.C)

    def din(name, shape, dt):
        return nc.dram_tensor(name, shape, dt, kind="ExternalInput")

    DN = ("td", "bu")
    xe_t = [din(f"xe_{n}", [128, TOTKT1[d] * 512], bf) for d, n in enumerate(DN)]
    m1_t = [din(f"m1_{n}", [128, TOTKT1[d] * 4], f32) for d, n in enumerate(DN)]
    i2_t = [din(f"i_{n}", [128, TOTKT2[d] * 8], i16) for d, n in enumerate(DN)]
    m2_t = [din(f"m2_{n}", [128, TOTKT2[d] * 2], f32) for d, n in enumerate(DN)]
    sf_t = [din(f"sf_{n}", [128, c.NB], f32) for d, n in enumerate(DN)]
    batch_t = din("batchcol", [128, c.NB], f32)
    mw_t = din("mw", [128, c.NB], f32)
    iota_t = din("iota512", [128, 512], f32)
    ident_t = din("ident", [128, 128], f32)
    ones_t = din("ones", [128, 1], f32)
    gmask_t = din("gmask", [128, 1], f32)
    w1on_t = [din(f"w1on_{n}", [128, 512], bf) for n in DN]
    w1tg_t = [din(f"w1tg_{n}", [128, 512], bf) for n in DN]
    w2on_t = [din(f"w2on_{n}", [128, 64], bf) for n in DN]
    w2tg_t = [din(f"w2tg_{n}", [128, 64], bf) for n in DN]
    b1c_t = [din(f"b1c_{n}", [128, 2], f32) for n in DN]
    b2_t = [din(f"b2_{n}", [128, 1], f32) for n in DN]
    loss_t = nc.dram_tensor("loss", [1, 1], f32, kind="ExternalOutput")

    with tile.TileContext(nc) as tc:
        with (
            tc.tile_pool(name="const", bufs=1) as cpool,
            tc.tile_pool(name="dram", bufs=1, space="DRAM") as dpool,
        ):
            z2own = [dpool.tile([c.OWNP, 128], bf, tag=f"z2own{d}",
                                name=f"z2own{d}") for d in range(2)]
            z2full = [dpool.tile([c.NPAD, 128], bf, addr_space="Shared",
                                 tag=f"z2full{d}", name=f"z2full{d}")
                      for d in range(2)]
            ar_in = dpool.tile([128, 260], f32, tag="arin", name="arin")
            ar_out = dpool.tile([128, 260], f32, addr_space="Shared",
                                tag="arout", name="arout")

            # ---------------- consts ----------------
            iotasb = cpool.tile([128, 512], f32)
            nc.sync.dma_start(out=iotasb[:], in_=iota_t[:, :])
            identsb = cpool.tile([128, 128], f32)
            nc.sync.dma_start(out=identsb[:], in_=ident_t[:, :])
            identbf = cpool.tile([128, 128], bf)
            nc.vector.tensor_copy(out=identbf[:], in_=identsb[:])
            onesb = cpool.tile([128, 1], f32)
            nc.sync.dma_start(out=onesb[:], in_=ones_t[:, :])
            gmsb = cpool.tile([128, 1], f32)
            nc.sync.dma_start(out=gmsb[:], in_=gmask_t[:, :])
            batchsb = cpool.tile([128, c.NB], f32)
            nc.sync.dma_start(out=batchsb[:], in_=batch_t[:, :])
            mwsb = cpool.tile([128, c.NB], f32)
            nc.sync.dma_start(out=mwsb[:], in_=mw_t[:, :])
            w1sb = [[cpool.tile([128, 512], bf, tag=f"w1_{d}{v}",
                                name=f"w1_{d}{v}") for v in range(2)]
                    for d in range(2)]
            w2sb = [[cpool.tile([128, 64], bf, tag=f"w2_{d}{v}",
                                name=f"w2_{d}{v}") for v in range(2)]
                    for d in range(2)]
            b1csb = [cpool.tile([128, 2], f32, tag=f"b1c_{d}", name=f"b1c_{d}")
                     for d in range(2)]
            b2sb = [cpool.tile([128, 1], f32, tag=f"b2_{d}", name=f"b2_{d}")
                    for d in range(2)]
            sfsb = [cpool.tile([128, c.NB], f32, tag=f"sf_{d}", name=f"sf_{d}")
                    for d in range(2)]
            for d in range(2):
                nc.sync.dma_start(out=w1sb[d][0][:], in_=w1on_t[d][:, :])
                nc.sync.dma_start(out=w1sb[d][1][:], in_=w1tg_t[d][:, :])
                nc.sync.dma_start(out=w2sb[d][0][:], in_=w2on_t[d][:, :])
                nc.sync.dma_start(out=w2sb[d][1][:], in_=w2tg_t[d][:, :])
                nc.sync.dma_start(out=b1csb[d][:], in_=b1c_t[d][:, :])
                nc.sync.dma_start(out=b2sb[d][:], in_=b2_t[d][:, :])
                nc.sync.dma_start(out=sfsb[d][:], in_=sf_t[d][:, :])

            # z2 stashes (bf16): stash[0] = z2_td then h2_td; stash[1] = z2_bu
            stash = [cpool.tile([128, c.NB * 128], bf, tag=f"st{d}",
                                name=f"st{d}") for d in range(2)]
            # wide cosine accumulators (col b <- block b, overwritten once)
            cdot = cpool.tile([128, c.NB], f32)
            cd2 = cpool.tile([128, c.NB], f32)
            cn1 = cpool.tile([128, c.NB], f32)
            cn1b = cpool.tile([128, c.NB], f32)
            cn2 = cpool.tile([128, c.NB], f32)
            cn2b = cpool.tile([128, c.NB], f32)

            # ================= L1 (per dir) ===========================
            def l1_dir(d):
                kt1 = KT1[d]
                kt1off = np.r_[0, np.cumsum(kt1)].astype(np.int64)
                with (
                    tc.tile_pool(name=f"xe{d}", bufs=2) as xep,
                    tc.tile_pool(name=f"mm{d}", bufs=2) as mmp,
                    tc.tile_pool(name=f"sg{d}", bufs=4) as sgp,
                    tc.tile_pool(name=f"fz{d}", bufs=2) as fzp,
                    tc.tile_pool(name=f"pA{d}", bufs=2, space="PSUM") as pap,
                    tc.tile_pool(name=f"pH{d}", bufs=1, space="PSUM") as php,
                    tc.tile_pool(name=f"pT{d}", bufs=1, space="PSUM") as ptp,
                ):
                    for g in range(c.NG):
                        b0 = g * c.GB1
                        blks = list(range(b0, min(b0 + c.GB1, c.NB)))
                        k0, k1 = int(kt1off[b0]), int(kt1off[blks[-1] + 1])
                        nkt = k1 - k0
                        xet = xep.tile([128, nkt * 512], bf, tag="xe", name="xe")
                        nc.sync.dma_start(
                            out=xet[:], in_=xe_t[d][:, k0 * 512:k1 * 512])
                        m1t = mmp.tile([128, nkt * 4], f32, tag="m1", name="m1")
                        nc.sync.dma_start(
                            out=m1t[:], in_=m1_t[d][:, k0 * 4:k1 * 4])
                        gw = len(blks) * 128
                        sbA = [fzp.tile([128, 4 * c.GB1 * 128], bf,
                                        tag=f"sbA{v}", name=f"sbA{v}")
                               for v in range(2)]
                        for bi, b in enumerate(blks):
                            psA = [pap.tile([128, 512], f32, tag=f"pA{v}",
                                            name=f"pA{v}") for v in range(2)]
                            for t in range(int(kt1[b])):
                                kt = int(kt1off[b]) + t - k0
                                st = [sgp.tile([128, 128], bf, tag=f"s{v}",
                                               name=f"s{v}") for v in range(2)]
                                for v in range(2):
                                    # S[p, j] = (iota[j]==lane[p]) * val_v[p]
                                    nc.gpsimd.tensor_scalar(
                                        out=st[v][:], in0=iotasb[:, 0:128],
                                        scalar1=m1t[:, kt * 4:kt * 4 + 1],
                                        scalar2=m1t[:, kt * 4 + 2 - v:
                                                    kt * 4 + 3 - v],
                                        op0=EQ, op1=MUL)
                                for v in range(2):
                                    for ch in range(4):
                                        nc.tensor.matmul(
                                            out=psA[v][:, ch * 128:
                                                       (ch + 1) * 128],
                                            lhsT=xet[:, kt * 512 + ch * 128:
                                                     kt * 512 + (ch + 1) * 128],
                                            rhs=st[v][:],
                                            start=(t == 0),
                                            stop=(t == int(kt1[b]) - 1))
                            # copy chunks into grouped bf16 layout (strided)
                            for v in range(2):
                                nc.vector.tensor_copy(
                                    out=sbA[v][:].rearrange(
                                        "p (ch n) -> p ch n",
                                        ch=4)[:, :, bi * 128:(bi + 1) * 128],
                                    in_=psA[v][:].rearrange(
                                        "p (ch n) -> p ch n", ch=4))
                        # ---- W1 apply (wide, feature-major) ----
                        hT = [php.tile([128, c.GB1 * 128], f32, tag=f"hT{v}",
                                       name=f"hT{v}") for v in range(2)]
                        for v in range(2):
                            for ch in range(4):
                                nc.tensor.matmul(
                                    out=hT[v][:, 0:gw],
                                    lhsT=w1sb[d][v][:, ch * 128:(ch + 1) * 128],
                                    rhs=sbA[v][:].rearrange(
                                        "p (ch n) -> p ch n", ch=4)[:, ch, 0:gw],
                                    start=(ch == 0), stop=(ch == 3))
                        # relu(h + b1): bias per feature = per partition
                        hsb = [fzp.tile([128, c.GB1 * 128], bf, tag=f"h{v}",
                                        name=f"h{v}") for v in range(2)]
                        for v in range(2):
                            nc.scalar.activation(
                                out=hsb[v][:, 0:gw], in_=hT[v][:, 0:gw],
                                func=AF.Relu, bias=b1csb[d][:, v:v + 1])
                        # ---- W2 apply: z2T [on64|tgt64, nodes] ----
                        z2T = ptp.tile([128, c.GB1 * 128], f32, tag="z2T",
                                       name="z2T")
                        for v in range(2):
                            nc.tensor.matmul(
                                out=z2T[v * 64:(v + 1) * 64, 0:gw],
                                lhsT=w2sb[d][v][:], rhs=hsb[v][:, 0:gw],
                                start=True, stop=True)
                        z2Tsb = fzp.tile([128, c.GB1 * 128], bf, tag="z2Tsb",
                                         name="z2Tsb")
                        nc.vector.tensor_copy(out=z2Tsb[:, 0:gw],
                                              in_=z2T[:, 0:gw])
                        for bi, b in enumerate(blks):
                            ztp = ptp.tile([128, 128], bf, tag="ztp",
                                           name="ztp")
                            nc.tensor.transpose(
                                out=ztp[:],
                                in_=z2Tsb[:, bi * 128:(bi + 1) * 128],
                                identity=identbf[:])
                            nc.vector.tensor_copy(
                                out=stash[d][:, b * 128:(b + 1) * 128],
                                in_=ztp[:])
                            nc.sync.dma_start(
                                out=z2own[d][b * 128:(b + 1) * 128, :],
                                in_=stash[d][:, b * 128:(b + 1) * 128])

            l1_dir(0)
            nc.gpsimd.collective_compute(
                "AllGather", mybir.AluOpType.bypass,
                replica_groups=[list(range(c.C))],
                ins=[z2own[0].opt()], outs=[z2full[0].opt()])
            l1_dir(1)
            nc.gpsimd.collective_compute(
                "AllGather", mybir.AluOpType.bypass,
                replica_groups=[list(range(c.C))],
                ins=[z2own[1].opt()], outs=[z2full[1].opt()])

            # ================= L2 (per dir) ===========================
            with tc.tile_pool(name="plps", bufs=1, space="PSUM") as plp:
                pool_ps = [plp.tile([128, 128], f32, tag=f"pl{d}",
                                    name=f"pl{d}") for d in range(2)]

                def l2_dir(d):
                    kt2, kt2off = KT2[d], KT2off[d]
                    wlen = lambda w: min(c.WIN, c.NPAD - w * c.WIN)
                    pool_mm = [0]
                    tot_pool = c.NB
                    with (
                        tc.tile_pool(name=f"gz{d}", bufs=2) as gzp,
                        tc.tile_pool(name=f"ix{d}", bufs=2) as ixp,
                        tc.tile_pool(name=f"mz{d}", bufs=2) as mzp,
                        tc.tile_pool(name=f"sz{d}", bufs=4) as szp,
                        tc.tile_pool(name=f"fh{d}", bufs=2) as fhp,
                        tc.tile_pool(name=f"pG{d}", bufs=2, space="PSUM") as pgp,
                        tc.tile_pool(name=f"pU{d}", bufs=2, space="PSUM") as pup,
                    ):
                        for sg, ops in enumerate(sched2[d]):
                            g4s = list(range(sg * c.SGF,
                                             min((sg + 1) * c.SGF, c.NG)))
                            if ops:
                                sk0 = ops[0][1]
                                sk1 = ops[-1][1] + ops[-1][2]
                            else:
                                sk0 = sk1 = 0
                            snk = max(sk1 - sk0, 1)
                            gzt = gzp.tile([128, snk * 128], bf, tag="gz",
                                           name="gz")
                            it = ixp.tile([128, snk * 8], i16, tag="ix",
                                          name="ix")
                            m2t = mzp.tile([128, snk * 2], f32, tag="m2",
                                           name="m2")
                            if sk1 > sk0:
                                nc.sync.dma_start(
                                    out=it[:, 0:(sk1 - sk0) * 8],
                                    in_=i2_t[d][:, sk0 * 8:sk1 * 8])
                                nc.sync.dma_start(
                                    out=m2t[:, 0:(sk1 - sk0) * 2],
                                    in_=m2_t[d][:, sk0 * 2:sk1 * 2])
                            for w, base, nk in ops:
                                o = base - sk0
                                nc.gpsimd.dma_gather(
                                    gzt[:, o * 128:(o + nk) * 128].rearrange(
                                        "p (k e) -> p k e", k=nk, e=128),
                                    z2full[d][w * c.WIN:w * c.WIN + wlen(w), :],
                                    it[:, o * 8:(o + nk) * 8],
                                    nk * 128, nk * 128, 128,
                                    elem_step=None, single_packet=False)
                            for g in g4s:
                                blks = list(range(g * c.GB1,
                                                  min((g + 1) * c.GB1, c.NB)))
                                gw = len(blks) * 128
                                h2T = pgp.tile([128, c.GB1 * 128], f32,
                                               tag="h2T", name="h2T")
                                nmm = int(kt2[g].sum()) + len(blks)
                                emitted = 0
                                # self-loop diag matmuls first (start=True
                                # initializes each block's column slice)
                                for bi, b in enumerate(blks):
                                    dg = szp.tile([128, 128], bf, tag="dg",
                                                  name="dg")
                                    nc.gpsimd.tensor_scalar(
                                        out=dg[:], in0=identsb[:],
                                        scalar1=sfsb[d][:, b:b + 1],
                                        scalar2=None, op0=MUL)
                                    nc.tensor.matmul(
                                        out=h2T[:, bi * 128:(bi + 1) * 128],
                                        lhsT=stash[d][:, b * 128:(b + 1) * 128],
                                        rhs=dg[:],
                                        start=True, stop=(emitted == nmm - 1),
                                        skip_group_check=True)
                                    emitted += 1
                                for w in range(c.NW2):
                                    for t in range(int(kt2[g, w])):
                                        ko = int(kt2off[g, w]) + t - sk0
                                        s2 = szp.tile([128, c.GB1 * 128], bf,
                                                      tag="s2", name="s2")
                                        nc.vector.tensor_scalar(
                                            out=s2[:], in0=iotasb[:],
                                            scalar1=m2t[:, ko * 2:ko * 2 + 1],
                                            scalar2=m2t[:, ko * 2 + 1:
                                                        ko * 2 + 2],
                                            op0=EQ, op1=MUL)
                                        nc.tensor.matmul(
                                            out=h2T[:, 0:gw],
                                            lhsT=gzt[:, ko * 128:(ko + 1) * 128],
                                            rhs=s2[:, 0:gw],
                                            start=False,
                                            stop=(emitted == nmm - 1),
                                            skip_group_check=True)
                                        emitted += 1
                                # bias per feature (partition) + to bf16
                                h2Tsb = fhp.tile([128, c.GB1 * 128], bf,
                                                 tag="h2Tsb", name="h2Tsb")
                                nc.vector.tensor_scalar(
                                    out=h2Tsb[:, 0:gw], in0=h2T[:, 0:gw],
                                    scalar1=b2sb[d][:], scalar2=None, op0=ADD)
                                for bi, b in enumerate(blks):
                                    htp = pup.tile([128, 128], bf, tag="htp",
                                                   name="htp")
                                    nc.tensor.transpose(
                                        out=htp[:],
                                        in_=h2Tsb[:, bi * 128:(bi + 1) * 128],
                                        identity=identbf[:])
                                    h2b = fhp.tile([128, 128], bf, tag="h2b",
                                                   name="h2b")
                                    nc.vector.tensor_copy(out=h2b[:],
                                                          in_=htp[:])
                                    pp = szp.tile([128, 128], bf, tag="pp",
                                                  name="pp")
                                    nc.gpsimd.tensor_scalar(
                                        out=pp[:], in0=iotasb[:, 0:128],
                                        scalar1=batchsb[:, b:b + 1],
                                        scalar2=None, op0=EQ)
                                    nc.tensor.matmul(
                                        out=pool_ps[d][:], lhsT=pp[:],
                                        rhs=h2b[:],
                                        start=(pool_mm[0] == 0),
                                        stop=(pool_mm[0] == tot_pool - 1),
                                        skip_group_check=True)
                                    pool_mm[0] += 1
                                    if d == 0:
                                        nc.vector.tensor_copy(
                                            out=stash[0][:,
                                                         b * 128:(b + 1) * 128],
                                            in_=h2b[:])
                                    else:
                                        td = stash[0][:, b * 128:(b + 1) * 128]
                                        sc = fhp.tile([128, 64], f32, tag="sc",
                                                      name="sc")
                                        sc2 = fhp.tile([128, 64], f32,
                                                       tag="sc2", name="sc2")
                                        nc.vector.scalar_tensor_tensor(
                                            out=sc[:], in0=td[:, 0:64],
                                            scalar=1.0, in1=td[:, 64:128],
                                            op0=MUL, op1=MUL,
                                            accum_out=cdot[:, b:b + 1])
                                        nc.vector.scalar_tensor_tensor(
                                            out=sc2[:], in0=h2b[:, 0:64],
                                            scalar=1.0, in1=h2b[:, 64:128],
                                            op0=MUL, op1=MUL,
                                            accum_out=cd2[:, b:b + 1])
                                        nc.vector.scalar_tensor_tensor(
                                            out=sc[:], in0=td[:, 0:64],
                                            scalar=1.0, in1=td[:, 0:64],
                                            op0=MUL, op1=MUL,
                                            accum_out=cn1[:, b:b + 1])
                                        nc.vector.scalar_tensor_tensor(
                                            out=sc2[:], in0=h2b[:, 0:64],
                                            scalar=1.0, in1=h2b[:, 0:64],
                                            op0=MUL, op1=MUL,
                                            accum_out=cn1b[:, b:b + 1])
                                        nc.vector.scalar_tensor_tensor(
                                            out=sc[:], in0=td[:, 64:128],
                                            scalar=1.0, in1=td[:, 64:128],
                                            op0=MUL, op1=MUL,
                                            accum_out=cn2[:, b:b + 1])
                                        nc.vector.scalar_tensor_tensor(
                                            out=sc2[:], in0=h2b[:, 64:128],
                                            scalar=1.0, in1=h2b[:, 64:128],
                                            op0=MUL, op1=MUL,
                                            accum_out=cn2b[:, b:b + 1])

                l2_dir(0)
                l2_dir(1)

                # ============ masked cosine tail + pool AR ============
                with (
                    tc.tile_pool(name="tail", bufs=2) as tlp,
                    tc.tile_pool(name="tps", bufs=2, space="PSUM") as tpp,
                ):
                    dot = tlp.tile([128, c.NB], f32, tag="dot", name="dot")
                    nc.vector.tensor_tensor(out=dot[:], in0=cdot[:],
                                            in1=cd2[:], op=ADD)
                    n1 = tlp.tile([128, c.NB], f32, tag="n1", name="n1")
                    nc.vector.tensor_tensor(out=n1[:], in0=cn1[:], in1=cn1b[:],
                                            op=ADD)
                    n2 = tlp.tile([128, c.NB], f32, tag="n2", name="n2")
                    nc.vector.tensor_tensor(out=n2[:], in0=cn2[:], in1=cn2b[:],
                                            op=ADD)

                    def rcp_sqrt(nt, tag):
                        r = tlp.tile([128, c.NB], f32, tag=tag, name=tag)
                        nc.scalar.sqrt(out=r[:], in_=nt[:])
                        nc.vector.tensor_scalar_max(out=r[:], in0=r[:],
                                                    scalar1=1e-12)
                        nc.vector.reciprocal(out=r[:], in_=r[:])
                        return r

                    r1 = rcp_sqrt(n1, "r1")
                    r2 = rcp_sqrt(n2, "r2")
                    cosv = tlp.tile([128, c.NB], f32, tag="cosv", name="cosv")
                    nc.vector.tensor_tensor(out=cosv[:], in0=dot[:], in1=r1[:],
                                            op=MUL)
                    nc.vector.tensor_tensor(out=cosv[:], in0=cosv[:],
                                            in1=r2[:], op=MUL)
                    term = tlp.tile([128, c.NB], f32, tag="term", name="term")
                    macc = tlp.tile([128, 1], f32, tag="macc", name="macc")
                    nc.vector.tensor_tensor(out=term[:], in0=mwsb[:],
                                            in1=cosv[:], op=MUL)
                    nc.vector.scalar_tensor_tensor(
                        out=term[:], in0=mwsb[:], scalar=1.0, in1=term[:],
                        op0=MUL, op1=SUB, accum_out=macc[:])
                    msps = tpp.tile([1, 1], f32, tag="ms", name="ms")
                    nc.tensor.matmul(out=msps[:], lhsT=macc[:], rhs=onesb[:],
                                     start=True, stop=True)

                    arsb = tlp.tile([128, 260], f32, tag="arsb", name="arsb")
                    nc.vector.memset(arsb[:], 0.0)
                    nc.vector.tensor_copy(out=arsb[:, 0:128],
                                          in_=pool_ps[0][:])
                    nc.vector.tensor_copy(out=arsb[:, 128:256],
                                          in_=pool_ps[1][:])
                    nc.vector.tensor_copy(out=arsb[0:1, 256:257], in_=msps[:])
                    nc.sync.dma_start(out=ar_in[:, :], in_=arsb[:])
                    nc.gpsimd.collective_compute(
                        "AllReduce", mybir.AluOpType.add,
                        replica_groups=[list(range(c.C))],
                        ins=[ar_in.opt()], outs=[ar_out.opt()])
                    ar2 = tlp.tile([128, 260], f32, tag="ar2", name="ar2")
                    nc.sync.dma_start(out=ar2[:], in_=ar_out[:, :])

                    # pooled cosine
                    def gacc(u0, u1, v0, v1, tag):
                        a1 = tlp.tile([128, 1], f32, tag=f"{tag}a",
                                      name=f"{tag}a")
                        a2 = tlp.tile([128, 1], f32, tag=f"{tag}b",
                                      name=f"{tag}b")
                        scr = tlp.tile([128, 64], f32, tag=f"{tag}s",
                                       name=f"{tag}s")
                        nc.vector.scalar_tensor_tensor(
                            out=scr[:], in0=u0, scalar=1.0, in1=v0,
                            op0=MUL, op1=MUL, accum_out=a1[:])
                        nc.vector.scalar_tensor_tensor(
                            out=scr[:], in0=u1, scalar=1.0, in1=v1,
                            op0=MUL, op1=MUL, accum_out=a2[:])
                        s = tlp.tile([128, 1], f32, tag=f"{tag}c",
                                     name=f"{tag}c")
                        nc.vector.tensor_tensor(out=s[:], in0=a1[:],
                                                in1=a2[:], op=ADD)
                        return s

                    tdon, tdtg = ar2[:, 0:64], ar2[:, 64:128]
                    buon, butg = ar2[:, 128:192], ar2[:, 192:256]
                    gdot = gacc(tdon, buon, tdtg, butg, "gd")
                    gn1 = gacc(tdon, buon, tdon, buon, "g1")
                    gn2 = gacc(tdtg, butg, tdtg, butg, "g2")

                    def rcp1(nt, tag):
                        r = tlp.tile([128, 1], f32, tag=tag, name=tag)
                        nc.scalar.sqrt(out=r[:], in_=nt[:])
                        nc.vector.tensor_scalar_max(out=r[:], in0=r[:],
                                                    scalar1=1e-12)
                        nc.vector.reciprocal(out=r[:], in_=r[:])
                        return r

                    gr1 = rcp1(gn1, "gr1")
                    gr2 = rcp1(gn2, "gr2")
                    cosg = tlp.tile([128, 1], f32, tag="cosg", name="cosg")
                    nc.vector.tensor_tensor(out=cosg[:], in0=gdot[:],
                                            in1=gr1[:], op=MUL)
                    nc.vector.tensor_tensor(out=cosg[:], in0=cosg[:],
                                            in1=gr2[:], op=MUL)
                    gterm = tlp.tile([128, 1], f32, tag="gt", name="gt")
                    nc.vector.tensor_scalar(out=gterm[:], in0=cosg[:],
                                            scalar1=-1.0, scalar2=1.0,
                                            op0=MUL, op1=ADD)
                    nc.vector.tensor_tensor(out=gterm[:], in0=gterm[:],
                                            in1=gmsb[:], op=MUL)
                    gsps = tpp.tile([1, 1], f32, tag="gs", name="gs")
                    nc.tensor.matmul(out=gsps[:], lhsT=gterm[:], rhs=onesb[:],
                                     start=True, stop=True)
                    l1t = tlp.tile([1, 1], f32, tag="l1", name="l1")
                    nc.scalar.activation(out=l1t[:], in_=gsps[:], func=AF.Copy,
                                         scale=1.0 / c.G)
                    l2t = tlp.tile([1, 1], f32, tag="l2", name="l2")
                    nc.scalar.activation(out=l2t[:], in_=ar2[0:1, 256:257],
                                         func=AF.Copy, scale=1.0 / c.M)
                    nc.vector.tensor_tensor(out=l1t[:], in0=l1t[:],
                                            in1=l2t[:], op=ADD)
                    nc.sync.dma_start(out=loss_t[:, :], in_=l1t[:])

    return nc


# ---------------------------------------------------------------- entry

LAST_RESULT = None


def kernel(_trace=False, **inputs):
    global LAST_RESULT
    import time
    from concourse import bass_utils
    cfg = FULL
    t0 = time.monotonic()
    meta, in_maps = host_prep(cfg, inputs)
    t1 = time.monotonic()
    nc = build_program(cfg, meta)
    t2 = time.monotonic()
    nc.compile()
    t3 = time.monotonic()
    res = bass_utils.run_bass_kernel_spmd(
        nc, in_maps, core_ids=list(range(cfg.C)),
        trace=_trace, trace_cores=[0] if _trace else None)
    t4 = time.monotonic()
    print(f"[kernel] prep {t1-t0:.1f}s build {t2-t1:.1f}s "
          f"compile {t3-t2:.1f}s run {t4-t3:.1f}s", file=sys.stderr)
    LAST_RESULT = res
    return np.float32(res.results[0]["loss"][0, 0])


# revision 11
# speedup vs baseline: 6.4719x; 3.0600x over previous
"""Trainium2 Bass kernel for the rumor-GCN masked-autoencoder loss (v4).

Strategy (8 NeuronCores, SPMD single NEFF):
  - Layer 1 message passing is HOST-PREGATHERED: for each core/direction the
    host materializes xe[slot] = x[src[slot]] in (dst-block, k-tile) slot
    order, shipped pre-tiled [128, KT*512] bf16, streamed with large
    sequential DMAs -- zero gather descriptors for layer 1.
  - All one-hot S matrices (GCN norms dinv[src]*dinv[dst] folded into the
    values; "on"-encoder masking folded by zeroing masked-src rows) are
    PRE-BUILT ON THE HOST and streamed as bf16 -- no on-chip generation.
  - L1 aggregation in normal form: A[node, 512] += S_v^T @ xe per k-tile
    (two wide matmuls); A transposed chunkwise on the PE; W1/W2 applied per
    4-block group with wide matmuls in feature-major form; z2 transposed
    back per block for the AllGather and stashed in SBUF.
  - Layer 2: z2 AllGathered (bf16); dma_gather per (8-block supergroup, 32K
    window) with (4-block group, window) k-tile buckets; aggregation with
    host-shipped narrow per-(tile, block) band S tiles into per-block
    PSUM accumulators (node-major, no transposes); self-loop terms from the
    SBUF z2 stash via host-shipped diag tiles (no gather slots).
  - Pooling via host-shipped batch-onehot matmuls into persistent PSUM; the
    masked-node cosine loss is computed per block inside the L2-BU loop
    (wide-tile tail), one small AllReduce finishes the scalar loss.
"""

import sys

import numpy as np

sys.path.insert(0, "/opt/trn_rl_repo")

# ---------------------------------------------------------------- config


class Cfg:
    def __init__(self, N, E, G, M, C=8, WIN=28672, GB1=4, SGF=2):
        self.N, self.E, self.G, self.M, self.C = N, E, G, M, C
        self.IN, self.HID, self.OUT = 512, 128, 64
        self.WIN = WIN
        self.GB1 = GB1          # blocks per psum group
        self.SGF = SGF          # L2: psum groups per gather supergroup
        assert N % C == 0
        self.OWN = N // C
        self.NB = -(-self.OWN // 128)
        self.OWNP = self.NB * 128
        self.NPAD = C * self.OWNP
        self.NW2 = -(-self.NPAD // WIN)
        self.NG = -(-self.NB // GB1)
        self.NSG = -(-self.NG // SGF)


FULL = Cfg(N=200000, E=400000, G=128, M=100000)

_WNAMES = [p + s for p in ("on_td", "on_bu", "tgt_td", "tgt_bu")
           for s in ("_W1", "_b1", "_W2", "_b2")]


def _rep16(idx_flat):
    n = len(idx_flat)
    assert n % 16 == 0
    blk = np.zeros((16, n // 16), dtype=np.int16)
    k = np.arange(n)
    blk[k % 16, k // 16] = idx_flat
    return np.tile(blk, (8, 1))


def _bcast(vec, parts=128):
    return np.broadcast_to(np.asarray(vec, np.float32)[None, :],
                           (parts, len(vec))).copy()


def _tile_rows(a, ncols):
    kt = a.shape[0] // 128
    return np.ascontiguousarray(
        a.reshape(kt, 128, ncols).transpose(1, 0, 2).reshape(128, kt * ncols))


# ---------------------------------------------------------------- host prep


def _assign_nodes(cfg, wtd, wbu):
    c = cfg
    w = wtd + wbu
    order = np.argsort(-w, kind="stable")
    core = np.empty(c.N, np.int64)
    grid = np.arange(c.N) % (2 * c.C)
    snake = np.where(grid < c.C, grid, 2 * c.C - 1 - grid)
    core[order] = snake
    blk = np.empty(c.N, np.int64)
    lane = np.empty(c.N, np.int64)
    for ci in range(c.C):
        nodes = order[core[order] == ci]
        nb = c.NB
        g = np.arange(len(nodes)) % (2 * nb)
        snb = np.where(g < nb, g, 2 * nb - 1 - g)
        blk[nodes] = snb
        ord2 = np.argsort(snb, kind="stable")
        srt = snb[ord2]
        start = np.r_[True, srt[1:] != srt[:-1]]
        segstart = np.maximum.accumulate(
            np.where(start, np.arange(len(srt)), 0))
        lane[nodes[ord2]] = np.arange(len(srt)) - segstart
    assert lane.max() < 128
    return core, blk, lane


def host_prep(cfg, inp):
    import ml_dtypes
    bf16 = ml_dtypes.bfloat16
    c = cfg
    x = np.asarray(inp["x"], np.float32)
    token = np.asarray(inp["enc_mask_token"], np.float32).reshape(-1)
    token_zero = not np.any(token)
    tokbf = token.astype(bf16)
    ei = np.asarray(inp["edge_index"])
    src, dst = ei[0].astype(np.int64), ei[1].astype(np.int64)
    batch = np.asarray(inp["batch"]).astype(np.int64)
    mask_nodes = np.asarray(inp["mask_nodes"]).astype(np.int64)
    W = {k: np.asarray(inp[k], np.float32) for k in _WNAMES}

    wtd = np.bincount(dst, minlength=c.N).astype(np.int64)
    wbu = np.bincount(src, minlength=c.N).astype(np.int64)
    dinv = [(1.0 / np.sqrt(wtd + 1.0)).astype(np.float32),
            (1.0 / np.sqrt(wbu + 1.0)).astype(np.float32)]
    mcount = np.bincount(mask_nodes, minlength=c.N).astype(np.float32)
    is_masked = mcount > 0
    xbf = x.astype(bf16)

    core, blk, lane = _assign_nodes(c, wtd, wbu)
    z2row = core * c.OWNP + blk * 128 + lane

    ed = {}
    for d in range(2):
        ad, asr = (dst, src) if d == 0 else (src, dst)
        val = dinv[d][ad] * dinv[d][asr]
        ed[d] = (ad, asr, val)

    all_nodes = np.arange(c.N, dtype=np.int64)
    e_full = {}
    for ci in range(c.C):
        own = core == ci
        on_nodes = all_nodes[own]
        for d in range(2):
            ad, asr, val = ed[d]
            sel = core[ad] == ci
            dl = np.concatenate([lane[ad[sel]], lane[on_nodes]])
            db = np.concatenate([blk[ad[sel]], blk[on_nodes]])
            sr = np.concatenate([asr[sel], on_nodes])
            vv = np.concatenate([val[sel], dinv[d][on_nodes] ** 2])
            vt = vv
            vo = vv * (~is_masked[sr])
            if not token_zero:
                msk = is_masked[sr]
                cacc = np.zeros((c.NB, 128), np.float32)
                np.add.at(cacc, (db[msk], dl[msk]), vv[msk])
                nz = np.nonzero(cacc)
                dl = np.concatenate([dl, nz[1]])
                db = np.concatenate([db, nz[0]])
                sr = np.concatenate([sr, np.full(len(nz[0]), -1, np.int64)])
                vt = np.concatenate([vt, np.zeros(len(nz[0]), np.float32)])
                vo = np.concatenate([vo, cacc[nz]])
            e_full[ci, d] = (dl, db, sr, vt, vo)

    cnt1 = np.zeros((2, c.C, c.NB), np.int64)
    for (ci, d), (dl, db, sr, vt, vo) in e_full.items():
        np.add.at(cnt1[d, ci], db, 1)
    KT1 = [np.maximum(1, -(-cnt1[d].max(axis=0) // 128)) for d in range(2)]
    KT1off = [np.r_[0, np.cumsum(KT1[d])].astype(np.int64) for d in range(2)]
    TOTKT1 = [int(KT1off[d][-1]) for d in range(2)]

    # Band structure: for each (dir) a list per sg:
    #   bandops: ordered [(ko_global, band_idx, blk)]
    # built identically for every core (tile/band LAYOUT is shared; band
    # CONTENT varies per core).  Bands are assigned where ANY core has slots;
    # per-core empty bands ship zero tiles.
    # To keep layout shared, band membership must be core-independent ->
    # derive from per-bucket per-block MAX counts.
    bandmeta = []   # [d] -> dict(nband, per_sg=[(blocks, ops=[(ko,band,blk)],
    #                                nmm={blk:count})])
    cnt2b = np.zeros((2, c.C, c.NG, c.NW2, c.GB1), np.int64)
    for d in range(2):
        ad, asr, _ = ed[d]
        np.add.at(cnt2b[d], (core[ad], blk[ad] // c.GB1,
                             z2row[asr] // c.WIN, blk[ad] % c.GB1), 1)
    # shared per-bucket block layout: use max over cores per (g,w,bi)
    blkmax = [cnt2b[d].max(axis=0) for d in range(2)]   # [NG, NW2, GB1]
    # within bucket, blocks packed in order bi=0..GB1-1, slot ranges from
    # blkmax; bucket capacity = KT2*128 (>= sum blkmax by construction? NO:
    # sum of per-block maxes can exceed 128*KT2). Recompute KT2 from blkmax.
    KT2 = [np.maximum(-(-blkmax[d].sum(axis=2) // 128),
                      (blkmax[d].sum(axis=2) > 0)) for d in range(2)]
    KT2off, sched2, TOTKT2 = [], [], []
    for d in range(2):
        off = np.zeros((c.NG, c.NW2), np.int64)
        acc = 0
        sgs = []
        for sg in range(c.NSG):
            g4s = range(sg * c.SGF, min((sg + 1) * c.SGF, c.NG))
            ops = []
            for w in range(c.NW2):
                nk = int(sum(KT2[d][g, w] for g in g4s))
                if nk == 0:
                    continue
                base = acc
                for g in g4s:
                    off[g, w] = acc
                    acc += KT2[d][g, w]
                ops.append((w, base, nk))
            sgs.append(ops)
        KT2off.append(off)
        sched2.append(sgs)
        TOTKT2.append(int(acc))

    for d in range(2):
        nband = 0
        per_sg = []
        for sg in range(c.NSG):
            g4s = list(range(sg * c.SGF, min((sg + 1) * c.SGF, c.NG)))
            opsl = []
            nmm = {}
            blocks = []
            for g in g4s:
                for bi in range(c.GB1):
                    b = g * c.GB1 + bi
                    if b < c.NB:
                        blocks.append(b)
                        nmm[b] = 1          # self matmul
            for g in g4s:
                for w in range(c.NW2):
                    if KT2[d][g, w] == 0:
                        continue
                    ko = int(KT2off[d][g, w])
                    # slot layout inside bucket: block runs of len blkmax
                    start_s = 0
                    for bi in range(c.GB1):
                        b = g * c.GB1 + bi
                        ln = int(blkmax[d][g, w, bi])
                        if ln == 0 or b >= c.NB:
                            start_s += ln
                            continue
                        t0, t1 = start_s // 128, (start_s + ln - 1) // 128
                        for t in range(t0, t1 + 1):
                            opsl.append((ko + t, nband, b))
                            nband += 1
                            nmm[b] += 1
                        start_s += ln
            per_sg.append(dict(blocks=blocks, ops=opsl, nmm=nmm))
        bandmeta.append(dict(nband=nband, per_sg=per_sg))

    # ---- shared weights etc.
    ident = np.eye(128, dtype=np.float32)
    w1 = {}
    w2 = {}
    b1c = {}
    b2bc = {}
    for d, nm in ((0, "td"), (1, "bu")):
        for v, pre in ((0, "on"), (1, "tgt")):
            wt = W[f"{pre}_{nm}_W1"]
            w1[d, v] = np.ascontiguousarray(
                wt.reshape(4, 128, 128).transpose(1, 0, 2).reshape(128, 512)
            ).astype(bf16)
            w2[d, v] = W[f"{pre}_{nm}_W2"].astype(bf16)
        b1c[d] = np.stack([W[f"on_{nm}_b1"], W[f"tgt_{nm}_b1"]],
                          axis=1).astype(np.float32)
        b2bc[d] = _bcast(np.concatenate([W[f"on_{nm}_b2"],
                                         W[f"tgt_{nm}_b2"]]))
    ones = np.ones((128, 1), np.float32)
    gmask = np.zeros((128, 1), np.float32)
    gmask[:c.G, 0] = 1.0

    # ---- per-core inputs
    in_maps = []
    for ci in range(c.C):
        own_sel = core == ci
        m = {}

        def nodecol(valarr, pad=0.0):
            a = np.full((128, c.NB), pad, np.float32)
            a[lane[own_sel], blk[own_sel]] = valarr[own_sel]
            return a

        m["mw"] = nodecol(mcount)

        # pooling one-hot [node, graph] per block; batch of pad lanes = -1
        bcol = nodecol(batch.astype(np.float32), pad=-1.0)
        pp = np.zeros((c.NB * 128, 128), np.float32)
        pl, pb = lane[own_sel], blk[own_sel]
        pp[pb * 128 + pl, batch[own_sel]] = 1.0
        m["ppool"] = _tile_rows(pp.astype(bf16), 128)

        for d, nm in ((0, "td"), (1, "bu")):
            # self-loop diag tiles
            sfv = nodecol(dinv[d] * dinv[d])
            sd = np.zeros((c.NB * 128, 128), np.float32)
            ll = np.arange(128)
            for b in range(c.NB):
                sd[b * 128 + ll, ll] = sfv[:, b]
            m[f"sd_{nm}"] = _tile_rows(sd.astype(bf16), 128)

            dl, db, sr, vt, vo = e_full[ci, d]
            # --- L1 slots ---
            order = np.argsort(db, kind="stable")
            sdb, sdl = db[order], dl[order]
            ssrc, svt, svo = sr[order], vt[order], vo[order]
            segchange = np.r_[True, sdb[1:] != sdb[:-1]]
            segstart = np.maximum.accumulate(
                np.where(segchange, np.arange(len(sdb)), 0))
            pos = np.arange(len(sdb)) - segstart
            slot = KT1off[d][sdb] * 128 + pos
            nslot1 = TOTKT1[d] * 128
            assert slot.max(initial=0) < nslot1
            xe = np.zeros((nslot1, 512), bf16)
            reg = ssrc >= 0
            xe[slot[reg]] = xbf[ssrc[reg]]
            if not token_zero:
                xe[slot[~reg]] = tokbf
            m[f"xe_{nm}"] = _tile_rows(xe, 512)
            # S tiles: [slot, 256] = [S_on | S_tgt]
            s1 = np.zeros((nslot1, 256), np.float32)
            s1[slot, sdl] = svo
            s1[slot, 128 + sdl] = svt
            m[f"s1_{nm}"] = _tile_rows(s1.astype(bf16), 256)

            # --- L2 slots: bucket (g4, w), block-run layout from blkmax ---
            ad, asr, val = ed[d]
            sel = core[ad] == ci
            f_db = blk[ad[sel]]
            f_g4 = f_db // c.GB1
            f_bi = f_db % c.GB1
            f_lane = lane[ad[sel]]
            f_row = z2row[asr[sel]]
            f_w = f_row // c.WIN
            f_rel = f_row - f_w * c.WIN
            f_val = val[sel]
            # run base offsets within bucket
            runoff = np.zeros((c.NG, c.NW2, c.GB1), np.int64)
            runoff[:, :, 1:] = np.cumsum(blkmax[d], axis=2)[:, :, :-1]
            # position within (g4, w, bi) run
            key = (f_g4 * c.NW2 + f_w) * c.GB1 + f_bi
            order = np.argsort(key, kind="stable")
            kk = key[order]
            segchange = np.r_[True, kk[1:] != kk[:-1]]
            segstart = np.maximum.accumulate(
                np.where(segchange, np.arange(len(kk)), 0))
            pos = np.arange(len(kk)) - segstart
            og4, ow, obi = f_g4[order], f_w[order], f_bi[order]
            slot = (KT2off[d][og4, ow] * 128 +
                    runoff[og4, ow, obi] + pos)
            nslot2 = TOTKT2[d] * 128
            assert slot.max(initial=0) < nslot2
            idx = np.zeros(nslot2, np.int64)
            idx[slot] = f_rel[order]
            m[f"i_{nm}"] = _rep16(idx.astype(np.int16))
            # band S tiles: [128, nband*128]
            bm = bandmeta[d]
            s2 = np.zeros((128, bm["nband"] * 128), np.float32)
            # compute each slot's band: need (ko_tile, block) -> band index
            band_of = {}
            for psg in bm["per_sg"]:
                for (ko, band, b) in psg["ops"]:
                    band_of[ko, b] = band
            okt = slot // 128
            opp = slot % 128
            oband = np.array([band_of[k, g * c.GB1 + bi]
                              for k, g, bi in zip(okt, og4, obi)])
            s2[opp, oband * 128 + f_lane[order]] = f_val[order]
            m[f"s2_{nm}"] = s2.astype(bf16)

            m[f"b2bc_{nm}"] = b2bc[d]
        m.update(ident=ident, ones=ones, gmask=gmask)
        for d, nm in ((0, "td"), (1, "bu")):
            m[f"w1on_{nm}"] = w1[d, 0]
            m[f"w1tg_{nm}"] = w1[d, 1]
            m[f"w2on_{nm}"] = w2[d, 0]
            m[f"w2tg_{nm}"] = w2[d, 1]
            m[f"b1c_{nm}"] = b1c[d]
        in_maps.append(m)

    meta = dict(KT1=KT1, TOTKT1=TOTKT1, KT2=KT2, KT2off=KT2off,
                sched2=sched2, TOTKT2=TOTKT2, bandmeta=bandmeta,
                assign=(core, blk, lane), z2row=z2row)
    return meta, in_maps


# ---------------------------------------------------------------- program


def build_program(cfg, meta):
    import concourse.bacc as bacc
    import concourse.mybir as mybir
    import concourse.tile as tile

    c = cfg
    KT1, TOTKT1 = meta["KT1"], meta["TOTKT1"]
    KT2, KT2off, TOTKT2 = meta["KT2"], meta["KT2off"], meta["TOTKT2"]
    sched2, bandmeta = meta["sched2"], meta["bandmeta"]
    f32, bf, i16 = mybir.dt.float32, mybir.dt.bfloat16, mybir.dt.int16
    MUL, ADD, SUB = (mybir.AluOpType.mult, mybir.AluOpType.add,
                     mybir.AluOpType.subtract)
    AF = mybir.ActivationFunctionType

    nc = bacc.Bacc("TRN2", target_bir_lowering=False, debug=False,
                   num_devices=c.C)

    def din(name, shape, dt):
        return nc.dram_tensor(name, shape, dt, kind="ExternalInput")

    DN = ("td", "bu")
    xe_t = [din(f"xe_{n}", [128, TOTKT1[d] * 512], bf) for d, n in enumerate(DN)]
    s1_t = [din(f"s1_{n}", [128, TOTKT1[d] * 256], bf) for d, n in enumerate(DN)]
    i2_t = [din(f"i_{n}", [128, TOTKT2[d] * 8], i16) for d, n in enumerate(DN)]
    s2_t = [din(f"s2_{n}", [128, bandmeta[d]["nband"] * 128], bf)
            for d, n in enumerate(DN)]
    sd_t = [din(f"sd_{n}", [128, c.NB * 128], bf) for d, n in enumerate(DN)]
    pp_t = din("ppool", [128, c.NB * 128], bf)
    mw_t = din("mw", [128, c.NB], f32)
    ident_t = din("ident", [128, 128], f32)
    ones_t = din("ones", [128, 1], f32)
    gmask_t = din("gmask", [128, 1], f32)
    w1on_t = [din(f"w1on_{n}", [128, 512], bf) for n in DN]
    w1tg_t = [din(f"w1tg_{n}", [128, 512], bf) for n in DN]
    w2on_t = [din(f"w2on_{n}", [128, 64], bf) for n in DN]
    w2tg_t = [din(f"w2tg_{n}", [128, 64], bf) for n in DN]
    b1c_t = [din(f"b1c_{n}", [128, 2], f32) for n in DN]
    b2bc_t = [din(f"b2bc_{n}", [128, 128], f32) for n in DN]
    loss_t = nc.dram_tensor("loss", [1, 1], f32, kind="ExternalOutput")

    with tile.TileContext(nc) as tc:
        with (
            tc.tile_pool(name="const", bufs=1) as cpool,
            tc.tile_pool(name="dram", bufs=1, space="DRAM") as dpool,
        ):
            z2own = [dpool.tile([c.OWNP, 128], bf, tag=f"z2own{d}",
                                name=f"z2own{d}") for d in range(2)]
            z2full = [dpool.tile([c.NPAD, 128], bf, addr_space="Shared",
                                 tag=f"z2full{d}", name=f"z2full{d}")
                      for d in range(2)]
            ar_in = dpool.tile([128, 260], f32, tag="arin", name="arin")
            ar_out = dpool.tile([128, 260], f32, addr_space="Shared",
                                tag="arout", name="arout")

            identsb = cpool.tile([128, 128], f32)
            nc.sync.dma_start(out=identsb[:], in_=ident_t[:, :])
            identbf = cpool.tile([128, 128], bf)
            nc.vector.tensor_copy(out=identbf[:], in_=identsb[:])
            onesb = cpool.tile([128, 1], f32)
            nc.sync.dma_start(out=onesb[:], in_=ones_t[:, :])
            gmsb = cpool.tile([128, 1], f32)
            nc.sync.dma_start(out=gmsb[:], in_=gmask_t[:, :])
            mwsb = cpool.tile([128, c.NB], f32)
            nc.sync.dma_start(out=mwsb[:], in_=mw_t[:, :])
            w1sb = [[cpool.tile([128, 512], bf, tag=f"w1_{d}{v}",
                                name=f"w1_{d}{v}") for v in range(2)]
                    for d in range(2)]
            w2sb = [[cpool.tile([128, 64], bf, tag=f"w2_{d}{v}",
                                name=f"w2_{d}{v}") for v in range(2)]
                    for d in range(2)]
            b1csb = [cpool.tile([128, 2], f32, tag=f"b1c_{d}", name=f"b1c_{d}")
                     for d in range(2)]
            b2sb = [cpool.tile([128, 128], f32, tag=f"b2_{d}", name=f"b2_{d}")
                    for d in range(2)]
            for d in range(2):
                nc.sync.dma_start(out=w1sb[d][0][:], in_=w1on_t[d][:, :])
                nc.sync.dma_start(out=w1sb[d][1][:], in_=w1tg_t[d][:, :])
                nc.sync.dma_start(out=w2sb[d][0][:], in_=w2on_t[d][:, :])
                nc.sync.dma_start(out=w2sb[d][1][:], in_=w2tg_t[d][:, :])
                nc.sync.dma_start(out=b1csb[d][:], in_=b1c_t[d][:, :])
                nc.sync.dma_start(out=b2sb[d][:], in_=b2bc_t[d][:, :])

            stash = [cpool.tile([128, c.NB * 128], bf, tag=f"st{d}",
                                name=f"st{d}") for d in range(2)]
            cdot = cpool.tile([128, c.NB], f32)
            cd2 = cpool.tile([128, c.NB], f32)
            cn1 = cpool.tile([128, c.NB], f32)
            cn1b = cpool.tile([128, c.NB], f32)
            cn2 = cpool.tile([128, c.NB], f32)
            cn2b = cpool.tile([128, c.NB], f32)

            # ================= L1 (per dir) ===========================
            def l1_dir(d):
                kt1 = KT1[d]
                kt1off = np.r_[0, np.cumsum(kt1)].astype(np.int64)
                with (
                    tc.tile_pool(name=f"xe{d}", bufs=2) as xep,
                    tc.tile_pool(name=f"s1p{d}", bufs=2) as s1p,
                    tc.tile_pool(name=f"fz{d}", bufs=2) as fzp,
                    tc.tile_pool(name=f"pA{d}", bufs=2, space="PSUM") as pap,
                    tc.tile_pool(name=f"pH{d}", bufs=1, space="PSUM") as php,
                    tc.tile_pool(name=f"pT{d}", bufs=1, space="PSUM") as ptp,
                ):
                    for g in range(c.NG):
                        b0 = g * c.GB1
                        blks = list(range(b0, min(b0 + c.GB1, c.NB)))
                        k0, k1 = int(kt1off[b0]), int(kt1off[blks[-1] + 1])
                        nkt = k1 - k0
                        xet = xep.tile([128, nkt * 512], bf, tag="xe",
                                       name="xe")
                        nc.sync.dma_start(
                            out=xet[:], in_=xe_t[d][:, k0 * 512:k1 * 512])
                        s1s = s1p.tile([128, nkt * 256], bf, tag="s1",
                                       name="s1")
                        nc.sync.dma_start(
                            out=s1s[:], in_=s1_t[d][:, k0 * 256:k1 * 256])
                        gw = len(blks) * 128
                        sbA = [fzp.tile([128, 4 * c.GB1 * 128], bf,
                                        tag=f"sbA{v}", name=f"sbA{v}")
                               for v in range(2)]
                        for bi, b in enumerate(blks):
                            psA = [pap.tile([128, 512], f32, tag=f"pA{v}",
                                            name=f"pA{v}") for v in range(2)]
                            for t in range(int(kt1[b])):
                                kt = int(kt1off[b]) + t - k0
                                for v in range(2):
                                    nc.tensor.matmul(
                                        out=psA[v][:],
                                        lhsT=s1s[:, kt * 256 + v * 128:
                                                 kt * 256 + (v + 1) * 128],
                                        rhs=xet[:, kt * 512:(kt + 1) * 512],
                                        start=(t == 0),
                                        stop=(t == int(kt1[b]) - 1))
                            # A -> bf16, transpose chunks, grouped layout
                            for v in range(2):
                                asb = fzp.tile([128, 512], bf, tag=f"as{v}",
                                               name=f"as{v}")
                                nc.scalar.copy(out=asb[:], in_=psA[v][:])
                                trt = ptp.tile([128, 512], bf, tag="tr",
                                               name="tr")
                                for ch in range(4):
                                    nc.tensor.transpose(
                                        out=trt[:, ch * 128:(ch + 1) * 128],
                                        in_=asb[:, ch * 128:(ch + 1) * 128],
                                        identity=identbf[:])
                                nc.vector.tensor_copy(
                                    out=sbA[v][:].rearrange(
                                        "p (ch n) -> p ch n",
                                        ch=4)[:, :, bi * 128:(bi + 1) * 128],
                                    in_=trt[:].rearrange(
                                        "p (ch n) -> p ch n", ch=4))
                        hT = [php.tile([128, c.GB1 * 128], f32, tag=f"hT{v}",
                                       name=f"hT{v}") for v in range(2)]
                        for v in range(2):
                            for ch in range(4):
                                nc.tensor.matmul(
                                    out=hT[v][:, 0:gw],
                                    lhsT=w1sb[d][v][:, ch * 128:(ch + 1) * 128],
                                    rhs=sbA[v][:].rearrange(
                                        "p (ch n) -> p ch n",
                                        ch=4)[:, ch, 0:gw],
                                    start=(ch == 0), stop=(ch == 3))
                        hsb = [fzp.tile([128, c.GB1 * 128], bf, tag=f"h{v}",
                                        name=f"h{v}") for v in range(2)]
                        for v in range(2):
                            nc.scalar.activation(
                                out=hsb[v][:, 0:gw], in_=hT[v][:, 0:gw],
                                func=AF.Relu, bias=b1csb[d][:, v:v + 1])
                        z2T = php.tile([128, c.GB1 * 128], f32, tag="z2T",
                                       name="z2T")
                        for v in range(2):
                            nc.tensor.matmul(
                                out=z2T[v * 64:(v + 1) * 64, 0:gw],
                                lhsT=w2sb[d][v][:], rhs=hsb[v][:, 0:gw],
                                start=True, stop=True)
                        z2Tsb = fzp.tile([128, c.GB1 * 128], bf, tag="z2Tsb",
                                         name="z2Tsb")
                        nc.vector.tensor_copy(out=z2Tsb[:, 0:gw],
                                              in_=z2T[:, 0:gw])
                        trz = ptp.tile([128, c.GB1 * 128], bf, tag="tr",
                                       name="trz")
                        for bi, b in enumerate(blks):
                            nc.tensor.transpose(
                                out=trz[:, bi * 128:(bi + 1) * 128],
                                in_=z2Tsb[:, bi * 128:(bi + 1) * 128],
                                identity=identbf[:])
                        for bi, b in enumerate(blks):
                            nc.vector.tensor_copy(
                                out=stash[d][:, b * 128:(b + 1) * 128],
                                in_=trz[:, bi * 128:(bi + 1) * 128])
                            nc.sync.dma_start(
                                out=z2own[d][b * 128:(b + 1) * 128, :],
                                in_=stash[d][:, b * 128:(b + 1) * 128])

            l1_dir(0)
            nc.gpsimd.collective_compute(
                "AllGather", mybir.AluOpType.bypass,
                replica_groups=[list(range(c.C))],
                ins=[z2own[0].opt()], outs=[z2full[0].opt()])
            l1_dir(1)
            nc.gpsimd.collective_compute(
                "AllGather", mybir.AluOpType.bypass,
                replica_groups=[list(range(c.C))],
                ins=[z2own[1].opt()], outs=[z2full[1].opt()])

            # ================= L2 (per dir) ===========================
            with tc.tile_pool(name="plps", bufs=1, space="PSUM") as plp:
                pool_ps = [plp.tile([128, 128], f32, tag=f"pl{d}",
                                    name=f"pl{d}") for d in range(2)]

                def l2_dir(d):
                    wlen = lambda w: min(c.WIN, c.NPAD - w * c.WIN)
                    pool_mm = [0]
                    tot_pool = c.NB
                    with (
                        tc.tile_pool(name=f"gz{d}", bufs=2) as gzp,
                        tc.tile_pool(name=f"ix{d}", bufs=2) as ixp,
                        tc.tile_pool(name=f"s2{d}", bufs=2) as s2p,
                        tc.tile_pool(name=f"sd{d}", bufs=2) as sdp,
                        tc.tile_pool(name=f"fh{d}", bufs=3) as fhp,
                        tc.tile_pool(name=f"pB{d}", bufs=2, space="PSUM") as pbp,
                    ):
                        for sg, ops in enumerate(sched2[d]):
                            bm = bandmeta[d]["per_sg"][sg]
                            blocks, bops, nmm = (bm["blocks"], bm["ops"],
                                                 bm["nmm"])
                            if ops:
                                sk0 = ops[0][1]
                                sk1 = ops[-1][1] + ops[-1][2]
                            else:
                                sk0 = sk1 = 0
                            snk = max(sk1 - sk0, 1)
                            if bops:
                                bd0 = bops[0][1]
                                bd1 = bops[-1][1] + 1
                            else:
                                bd0 = bd1 = 0
                            nbd = max(bd1 - bd0, 1)
                            gzt = gzp.tile([128, snk * 128], bf, tag="gz",
                                           name="gz")
                            it = ixp.tile([128, snk * 8], i16, tag="ix",
                                          name="ix")
                            s2s = s2p.tile([128, nbd * 128], bf, tag="s2",
                                           name="s2")
                            sds = sdp.tile([128, len(blocks) * 128], bf,
                                           tag="sd", name="sd")
                            b00 = blocks[0]
                            nc.sync.dma_start(
                                out=sds[:],
                                in_=sd_t[d][:, b00 * 128:
                                            (b00 + len(blocks)) * 128])
                            pps = sdp.tile([128, len(blocks) * 128], bf,
                                           tag="pp", name="pp")
                            nc.sync.dma_start(
                                out=pps[:],
                                in_=pp_t[:, b00 * 128:
                                         (b00 + len(blocks)) * 128])
                            if sk1 > sk0:
                                nc.sync.dma_start(
                                    out=it[:, 0:(sk1 - sk0) * 8],
                                    in_=i2_t[d][:, sk0 * 8:sk1 * 8])
                            if bd1 > bd0:
                                nc.sync.dma_start(
                                    out=s2s[:, 0:(bd1 - bd0) * 128],
                                    in_=s2_t[d][:, bd0 * 128:bd1 * 128])
                            for w, base, nk in ops:
                                o = base - sk0
                                nc.gpsimd.dma_gather(
                                    gzt[:, o * 128:(o + nk) * 128].rearrange(
                                        "p (k e) -> p k e", k=nk, e=128),
                                    z2full[d][w * c.WIN:w * c.WIN + wlen(w), :],
                                    it[:, o * 8:(o + nk) * 8],
                                    nk * 128, nk * 128, 128,
                                    elem_step=None, single_packet=False)
                            # per-block psum slices of 2 group tiles:
                            # self first, then bands
                            psG = [pbp.tile([128, c.GB1 * 128], f32,
                                            tag=f"psG{j}", name=f"psG{j}")
                                   for j in range(c.SGF)]

                            def bslice(b):
                                gi = (b // c.GB1) % c.SGF
                                bi = b % c.GB1
                                return psG[gi][:, bi * 128:(bi + 1) * 128]

                            done = {}
                            for bi, b in enumerate(blocks):
                                nc.tensor.matmul(
                                    out=bslice(b),
                                    lhsT=sds[:, bi * 128:(bi + 1) * 128],
                                    rhs=stash[d][:, b * 128:(b + 1) * 128],
                                    start=True, stop=(nmm[b] == 1),
                                    skip_group_check=True)
                                done[b] = 1
                            for (ko, band, b) in bops:
                                nc.tensor.matmul(
                                    out=bslice(b),
                                    lhsT=s2s[:, (band - bd0) * 128:
                                             (band - bd0 + 1) * 128],
                                    rhs=gzt[:, (ko - sk0) * 128:
                                            (ko - sk0 + 1) * 128],
                                    start=False,
                                    stop=(done[b] + 1 == nmm[b]),
                                    skip_group_check=True)
                                done[b] += 1
                            # finalize per block
                            for bi, b in enumerate(blocks):
                                h2b = fhp.tile([128, 128], bf, tag="h2b",
                                               name="h2b")
                                nc.vector.tensor_tensor(
                                    out=h2b[:], in0=bslice(b),
                                    in1=b2sb[d][:], op=ADD)
                                nc.tensor.matmul(
                                    out=pool_ps[d][:],
                                    lhsT=pps[:, bi * 128:(bi + 1) * 128],
                                    rhs=h2b[:],
                                    start=(pool_mm[0] == 0),
                                    stop=(pool_mm[0] == tot_pool - 1),
                                    skip_group_check=True)
                                pool_mm[0] += 1
                                if d == 0:
                                    nc.vector.tensor_copy(
                                        out=stash[0][:, b * 128:(b + 1) * 128],
                                        in_=h2b[:])
                                else:
                                    td = stash[0][:, b * 128:(b + 1) * 128]
                                    sc = fhp.tile([128, 64], f32, tag="sc",
                                                  name="sc")
                                    sc2 = fhp.tile([128, 64], f32, tag="sc2",
                                                   name="sc2")
                                    nc.vector.scalar_tensor_tensor(
                                        out=sc[:], in0=td[:, 0:64], scalar=1.0,
                                        in1=td[:, 64:128], op0=MUL, op1=MUL,
                                        accum_out=cdot[:, b:b + 1])
                                    nc.vector.scalar_tensor_tensor(
                                        out=sc2[:], in0=h2b[:, 0:64],
                                        scalar=1.0, in1=h2b[:, 64:128],
                                        op0=MUL, op1=MUL,
                                        accum_out=cd2[:, b:b + 1])
                                    nc.vector.scalar_tensor_tensor(
                                        out=sc[:], in0=td[:, 0:64], scalar=1.0,
                                        in1=td[:, 0:64], op0=MUL, op1=MUL,
                                        accum_out=cn1[:, b:b + 1])
                                    nc.vector.scalar_tensor_tensor(
                                        out=sc2[:], in0=h2b[:, 0:64],
                                        scalar=1.0, in1=h2b[:, 0:64],
                                        op0=MUL, op1=MUL,
                                        accum_out=cn1b[:, b:b + 1])
                                    nc.vector.scalar_tensor_tensor(
                                        out=sc[:], in0=td[:, 64:128],
                                        scalar=1.0, in1=td[:, 64:128],
                                        op0=MUL, op1=MUL,
                                        accum_out=cn2[:, b:b + 1])
                                    nc.vector.scalar_tensor_tensor(
                                        out=sc2[:], in0=h2b[:, 64:128],
                                        scalar=1.0, in1=h2b[:, 64:128],
                                        op0=MUL, op1=MUL,
                                        accum_out=cn2b[:, b:b + 1])

                l2_dir(0)
                l2_dir(1)

                # ============ masked cosine tail + pool AR ============
                with (
                    tc.tile_pool(name="tail", bufs=2) as tlp,
                    tc.tile_pool(name="tps", bufs=2, space="PSUM") as tpp,
                ):
                    dot = tlp.tile([128, c.NB], f32, tag="dot", name="dot")
                    nc.vector.tensor_tensor(out=dot[:], in0=cdot[:],
                                            in1=cd2[:], op=ADD)
                    n1 = tlp.tile([128, c.NB], f32, tag="n1", name="n1")
                    nc.vector.tensor_tensor(out=n1[:], in0=cn1[:], in1=cn1b[:],
                                            op=ADD)
                    n2 = tlp.tile([128, c.NB], f32, tag="n2", name="n2")
                    nc.vector.tensor_tensor(out=n2[:], in0=cn2[:], in1=cn2b[:],
                                            op=ADD)

                    def rcp_sqrt(nt, tag):
                        r = tlp.tile([128, c.NB], f32, tag=tag, name=tag)
                        nc.scalar.sqrt(out=r[:], in_=nt[:])
                        nc.vector.tensor_scalar_max(out=r[:], in0=r[:],
                                                    scalar1=1e-12)
                        nc.vector.reciprocal(out=r[:], in_=r[:])
                        return r

                    r1 = rcp_sqrt(n1, "r1")
                    r2 = rcp_sqrt(n2, "r2")
                    cosv = tlp.tile([128, c.NB], f32, tag="cosv", name="cosv")
                    nc.vector.tensor_tensor(out=cosv[:], in0=dot[:], in1=r1[:],
                                            op=MUL)
                    nc.vector.tensor_tensor(out=cosv[:], in0=cosv[:],
                                            in1=r2[:], op=MUL)
                    term = tlp.tile([128, c.NB], f32, tag="term", name="term")
                    macc = tlp.tile([128, 1], f32, tag="macc", name="macc")
                    nc.vector.tensor_tensor(out=term[:], in0=mwsb[:],
                                            in1=cosv[:], op=MUL)
                    nc.vector.scalar_tensor_tensor(
                        out=term[:], in0=mwsb[:], scalar=1.0, in1=term[:],
                        op0=MUL, op1=SUB, accum_out=macc[:])
                    msps = tpp.tile([1, 1], f32, tag="ms", name="ms")
                    nc.tensor.matmul(out=msps[:], lhsT=macc[:], rhs=onesb[:],
                                     start=True, stop=True)

                    arsb = tlp.tile([128, 260], f32, tag="arsb", name="arsb")
                    nc.vector.memset(arsb[:], 0.0)
                    nc.vector.tensor_copy(out=arsb[:, 0:128],
                                          in_=pool_ps[0][:])
                    nc.vector.tensor_copy(out=arsb[:, 128:256],
                                          in_=pool_ps[1][:])
                    nc.vector.tensor_copy(out=arsb[0:1, 256:257], in_=msps[:])
                    nc.sync.dma_start(out=ar_in[:, :], in_=arsb[:])
                    nc.gpsimd.collective_compute(
                        "AllReduce", mybir.AluOpType.add,
                        replica_groups=[list(range(c.C))],
                        ins=[ar_in.opt()], outs=[ar_out.opt()])
                    ar2 = tlp.tile([128, 260], f32, tag="ar2", name="ar2")
                    nc.sync.dma_start(out=ar2[:], in_=ar_out[:, :])

                    def gacc(u0, u1, v0, v1, tag):
                        a1 = tlp.tile([128, 1], f32, tag=f"{tag}a",
                                      name=f"{tag}a")
                        a2 = tlp.tile([128, 1], f32, tag=f"{tag}b",
                                      name=f"{tag}b")
                        scr = tlp.tile([128, 64], f32, tag=f"{tag}s",
                                       name=f"{tag}s")
                        nc.vector.scalar_tensor_tensor(
                            out=scr[:], in0=u0, scalar=1.0, in1=v0,
                            op0=MUL, op1=MUL, accum_out=a1[:])
                        nc.vector.scalar_tensor_tensor(
                            out=scr[:], in0=u1, scalar=1.0, in1=v1,
                            op0=MUL, op1=MUL, accum_out=a2[:])
                        s = tlp.tile([128, 1], f32, tag=f"{tag}c",
                                     name=f"{tag}c")
                        nc.vector.tensor_tensor(out=s[:], in0=a1[:],
                                                in1=a2[:], op=ADD)
                        return s

                    tdon, tdtg = ar2[:, 0:64], ar2[:, 64:128]
                    buon, butg = ar2[:, 128:192], ar2[:, 192:256]
                    gdot = gacc(tdon, buon, tdtg, butg, "gd")
                    gn1 = gacc(tdon, buon, tdon, buon, "g1")
                    gn2 = gacc(tdtg, butg, tdtg, butg, "g2")

                    def rcp1(nt, tag):
                        r = tlp.tile([128, 1], f32, tag=tag, name=tag)
                        nc.scalar.sqrt(out=r[:], in_=nt[:])
                        nc.vector.tensor_scalar_max(out=r[:], in0=r[:],
                                                    scalar1=1e-12)
                        nc.vector.reciprocal(out=r[:], in_=r[:])
                        return r

                    gr1 = rcp1(gn1, "gr1")
                    gr2 = rcp1(gn2, "gr2")
                    cosg = tlp.tile([128, 1], f32, tag="cosg", name="cosg")
                    nc.vector.tensor_tensor(out=cosg[:], in0=gdot[:],
                                            in1=gr1[:], op=MUL)
                    nc.vector.tensor_tensor(out=cosg[:], in0=cosg[:],
                                            in1=gr2[:], op=MUL)
                    gterm = tlp.tile([128, 1], f32, tag="gt", name="gt")
                    nc.vector.tensor_scalar(out=gterm[:], in0=cosg[:],
                                            scalar1=-1.0, scalar2=1.0,
                                            op0=MUL, op1=ADD)
                    nc.vector.tensor_tensor(out=gterm[:], in0=gterm[:],
                                            in1=gmsb[:], op=MUL)
                    gsps = tpp.tile([1, 1], f32, tag="gs", name="gs")
                    nc.tensor.matmul(out=gsps[:], lhsT=gterm[:], rhs=onesb[:],
                                     start=True, stop=True)
                    l1t = tlp.tile([1, 1], f32, tag="l1", name="l1")
                    nc.scalar.activation(out=l1t[:], in_=gsps[:], func=AF.Copy,
                                         scale=1.0 / c.G)
                    l2t = tlp.tile([1, 1], f32, tag="l2", name="l2")
                    nc.scalar.activation(out=l2t[:], in_=ar2[0:1, 256:257],
                                         func=AF.Copy, scale=1.0 / c.M)
                    nc.vector.tensor_tensor(out=l1t[:], in0=l1t[:],
                                            in1=l2t[:], op=ADD)
                    nc.sync.dma_start(out=loss_t[:, :], in_=l1t[:])

    return nc


# ---------------------------------------------------------------- entry

LAST_RESULT = None


def kernel(_trace=False, **inputs):
    global LAST_RESULT
    import time
    from concourse import bass_utils
    cfg = FULL
    t0 = time.monotonic()
    meta, in_maps = host_prep(cfg, inputs)
    t1 = time.monotonic()
    nc = build_program(cfg, meta)
    t2 = time.monotonic()
    nc.compile()
    t3 = time.monotonic()
    res = bass_utils.run_bass_kernel_spmd(
        nc, in_maps, core_ids=list(range(cfg.C)),
        trace=_trace, trace_cores=[0] if _trace else None)
    t4 = time.monotonic()
    print(f"[kernel] prep {t1-t0:.1f}s build {t2-t1:.1f}s "
          f"compile {t3-t2:.1f}s run {t4-t3:.1f}s", file=sys.stderr)
    LAST_RESULT = res
    return np.float32(res.results[0]["loss"][0, 0])
